# revision 1
# baseline (speedup 1.0000x reference)
"""Channel-attention (XCA-style) Trainium2 kernel, 8-way SPMD.

Shapes (hardcoded): B=4, N=16384, D=256, H=2 heads, c=128.
Sharding: core ci -> batch b=ci//2, token half ci%2 (T=8192 tokens/core).
The q@k^T contraction reduces over N, so each pair of cores all-reduces the
small per-head Gram matrices plus correction side-products and L2 norm pieces.

Per-core pipeline (fp32 I/O, bf16 matmul operands):
  - SWDGE cast-DMA loads raw token-major chunks (fp32->bf16)
  - LN is never materialized: folded into weights (host) + per-token scalars;
    centering/bias corrections are applied to the c x c Grams analytically
  - xbar DMA-transpose -> d-major tiles; PE matmuls; Gram + thin-row
    accumulation; pair AllReduce overlapped with the v matmul pass
  - softmax on 128x128 logits; attn @ v; Wo matmul; all v/bias corrections
    folded into the final evacuation
"""
import sys, types

sys.path.insert(0, "/opt/trn_rl_repo")

try:
    import antenv
    if "antenv.axon_hooks" not in sys.modules:
        _hooks = types.ModuleType("antenv.axon_hooks")
        _hooks._hook = None
        _hooks.set_axon_ntff_profile_hook = lambda h: setattr(_hooks, "_hook", h)
        _hooks.get_axon_ntff_profile_hook = lambda: _hooks._hook
        sys.modules["antenv.axon_hooks"] = _hooks
        antenv.axon_hooks = _hooks
        from trn_agent_boot.trn_boot import _ntff_profile_via_ctypes
        _hooks.set_axon_ntff_profile_hook(
            _ntff_profile_via_ctypes("/opt/axon/libaxon_pjrt.so"))
except Exception:
    pass

import numpy as np
import ml_dtypes

import concourse.bass as bass
import concourse.bacc as bacc
import concourse.mybir as mybir
import concourse.tile as tile
from concourse.bass_utils import run_bass_kernel_spmd

BF16 = ml_dtypes.bfloat16
F32 = mybir.dt.float32
BF = mybir.dt.bfloat16
AL = mybir.AluOpType
AF = mybir.ActivationFunctionType
AX = mybir.AxisListType

B, N, D, H = 4, 16384, 256, 2
C = D // H
T = N // 2
NT = T // 128              # 64 token tiles / core
EPS_LN = 1e-5
EPS_NORM = 1e-12
N_CORES = 8
CHT = 8                    # token tiles per chunk
NCH = NT // CHT            # 4 chunks
PACKW = 784                # collective pack width

_nc_cache = {}


def _bcast(ap, rows=128):
    """Broadcast (partition-step-0) a single-row AP across `rows` partitions."""
    return bass.AP(tensor=ap.tensor, offset=ap.offset,
                   ap=[[0, rows]] + [list(x) for x in ap.ap[1:]])


def _build_nc():
    nc = bacc.Bacc("TRN2", target_bir_lowering=False, debug=False,
                   num_devices=N_CORES)

    def ein(name, shape, dt=F32):
        return nc.dram_tensor(name, list(shape), dt, kind="ExternalInput")

    d_x = ein("x_r", [T, D])            # kv source shard (input_R)
    d_y = ein("x_s", [T, D])            # q source shard (input_S)
    d_wq = ein("wqT", [D, D], BF)       # [d, o] LN-folded
    d_wk = ein("wkT", [D, D], BF)
    d_wv = ein("wvT", [D, D], BF)
    d_wo = ein("woT", [D, D], BF)
    d_svc = ein("sv_col", [D, 1], BF)
    d_bvc = ein("bv2_col", [D, 1], BF)
    d_sqc = ein("sq_col", [128, H])
    d_bqc = ein("bq2_col", [128, H])
    d_skc = ein("sk_col", [128, H])
    d_bkc = ein("bk2_col", [128, H])
    d_skr = ein("sk_row", [1, D])
    d_bkr = ein("bk2_row", [1, D])
    d_bor = ein("bo_row", [1, D])
    d_eye = ein("eye", [128, 128])
    d_temp = ein("temp", [1, H])
    d_out = nc.dram_tensor("out", [T, D], F32, kind="ExternalOutput")

    xv = d_x.rearrange("(j p) d -> p j d", p=128)
    yv = d_y.rearrange("(j p) d -> p j d", p=128)
    outv = d_out.rearrange("(j p) d -> p j d", p=128)

    with tile.TileContext(nc) as tc:
        import contextlib
        with contextlib.ExitStack() as ctx:
            _body(ctx, tc, nc, xv, yv, outv, d_wq, d_wk, d_wv, d_wo,
                  d_svc, d_bvc, d_sqc, d_bqc, d_skc, d_bkc, d_skr, d_bkr,
                  d_bor, d_eye, d_temp)
    nc.finalize()
    return nc


def _body(ctx, tc, nc, xv, yv, outv, d_wq, d_wk, d_wv, d_wo, d_svc, d_bvc,
          d_sqc, d_bqc, d_skc, d_bkc, d_skr, d_bkr, d_bor, d_eye, d_temp):
    E = ctx.enter_context
    consts = E(tc.tile_pool(name="consts", bufs=1))
    stats = E(tc.tile_pool(name="stats", bufs=1))
    stage = E(tc.tile_pool(name="stage", bufs=2))
    xtp = E(tc.tile_pool(name="xtp", bufs=2))
    qkp = E(tc.tile_pool(name="qkp", bufs=2))
    pers = E(tc.tile_pool(name="pers", bufs=1))
    post = E(tc.tile_pool(name="post", bufs=1))
    small = E(tc.tile_pool(name="small", bufs=4))
    outp = E(tc.tile_pool(name="outp", bufs=2))
    dram = E(tc.tile_pool(name="dram", bufs=1, space="DRAM"))
    accps = E(tc.tile_pool(name="accps", bufs=1, space="PSUM"))

    # ---------------- constants ----------------
    wq_sb = consts.tile([128, 2, D], BF, tag="wq")
    wk_sb = consts.tile([128, 2, D], BF, tag="wk")
    wv_sb = consts.tile([128, 2, D], BF, tag="wv")
    wo_sb = consts.tile([128, 2, D], BF, tag="wo")
    for dst, src in ((wq_sb, d_wq), (wk_sb, d_wk), (wv_sb, d_wv), (wo_sb, d_wo)):
        nc.sync.dma_start(out=dst[:], in_=src.rearrange("(h p) o -> p h o", p=128))
    sv_col = consts.tile([128, 2, 1], BF, tag="svc")
    bv_col = consts.tile([128, 2, 1], BF, tag="bvc")
    nc.sync.dma_start(out=sv_col[:], in_=d_svc.rearrange("(h p) o -> p h o", p=128))
    nc.sync.dma_start(out=bv_col[:], in_=d_bvc.rearrange("(h p) o -> p h o", p=128))
    sq_col = consts.tile([128, H], F32, tag="sqc")
    bq_col = consts.tile([128, H], F32, tag="bqc")
    sk_col = consts.tile([128, H], F32, tag="skc")
    bk_col = consts.tile([128, H], F32, tag="bkc")
    for dst, src in ((sq_col, d_sqc), (bq_col, d_bqc), (sk_col, d_skc),
                     (bk_col, d_bkc)):
        nc.sync.dma_start(out=dst[:], in_=src[:, :])
    skr_b = consts.tile([128, D], F32, tag="skrb")
    bkr_b = consts.tile([128, D], F32, tag="bkrb")
    nc.sync.dma_start(out=skr_b[:], in_=_bcast(d_skr[:, :]))
    nc.sync.dma_start(out=bkr_b[:], in_=_bcast(d_bkr[:, :]))
    bo_row = consts.tile([1, D], F32, tag="bor")
    nc.sync.dma_start(out=bo_row[:], in_=d_bor[:, :])
    eye_sb = consts.tile([128, 128], F32, tag="eye")
    nc.sync.dma_start(out=eye_sb[:], in_=d_eye[:, :])
    temp_b = consts.tile([128, H], F32, tag="tempb")
    nc.sync.dma_start(out=temp_b[:], in_=_bcast(d_temp[:, :]))
    ones_bf = consts.tile([128, 1], BF, tag="ones")
    nc.vector.memset(ones_bf[:], 1.0)
    epsln = consts.tile([128, 1], F32, tag="epsln")
    nc.vector.memset(epsln[:], EPS_LN)
    zb = consts.tile([128, 1], F32, tag="zb")
    nc.vector.memset(zb[:], 0.0)

    # ---------------- stats state ----------------
    ssq_r = stats.tile([128, NT], F32, tag="ssq_r")
    ssq_s = stats.tile([128, NT], F32, tag="ssq_s")
    invs_r = stats.tile([128, NT], F32, tag="invs_r")
    invs_s = stats.tile([128, NT], F32, tag="invs_s")
    arn = stats.tile([128, NT], F32, tag="arn")        # -aR = -muR*invsR (f32)
    wcols = stats.tile([128, NT, 3], BF, tag="wcols")  # [-aS, -aR, 1]

    nc.vector.memset(wcols[:, :, 2], 1.0)
    sq_scr = stats.tile([128, 256], F32, tag="sq_scr")

    xtr_all = pers.tile([128, NT, 2, 128], BF, tag="xtr")
    vt_all = pers.tile([128, 2, T], BF, tag="vt")

    acc = accps.tile([128, 1024], F32, tag="acc")
    # acc cols: Gt h0 0:128 h1 128:256 | Hqq 256:512 | Hkk 512:768
    # thinQ [0:3, 768:1024] thinK [4:7, 768:1024] Sc [8:11, 768:771]

    # ================= phase 1: stream chunks =================
    with tc.tile_pool(name="qkps", bufs=2, space="PSUM") as qkps, \
         tc.tile_pool(name="sumps", bufs=2, space="PSUM") as sumps:
        for ch in range(NCH):
            j0 = ch * CHT
            mu_rows = stage.tile([16, CHT * 128], BF, tag="mu_rows")
            mus_row = stage.tile([1, CHT * 128], BF, tag="mus_row")
            nc.gpsimd.memset(mu_rows[:, :], 0.0)
            xr_tm = stage.tile([128, CHT, D], BF, tag="xr_tm")
            ys_tm = stage.tile([128, CHT, D], BF, tag="ys_tm")
            nc.gpsimd.dma_start(out=xr_tm[:], in_=xv[:, j0:j0 + CHT, :])
            nc.gpsimd.dma_start(out=ys_tm[:], in_=yv[:, j0:j0 + CHT, :])

            for jj in range(CHT):
                j = j0 + jj
                nc.vector.scalar_tensor_tensor(
                    out=sq_scr[:], in0=xr_tm[:, jj, :], scalar=0.0,
                    op0=AL.bypass, op1=AL.mult, in1=xr_tm[:, jj, :],
                    accum_out=ssq_r[:, j:j + 1])
                nc.vector.scalar_tensor_tensor(
                    out=sq_scr[:], in0=ys_tm[:, jj, :], scalar=0.0,
                    op0=AL.bypass, op1=AL.mult, in1=ys_tm[:, jj, :],
                    accum_out=ssq_s[:, j:j + 1])

            # d-major transposes (xbar): out[p, e, t] = in[t, e*128+p]
            nc.sync.dma_start_transpose(xtr_all[:, j0:j0 + CHT, :, :], xr_tm[:])
            ytr = xtp.tile([128, CHT, 2, 128], BF, tag="ytr")
            nc.sync.dma_start_transpose(ytr[:], ys_tm[:])

            # means via PE ones-matmuls (rows 0=R, 1=S), 512-token groups
            for g in range(CHT // 4):
                sps = sumps.tile([1, 1024], F32, tag="sums")
                for q4 in range(4):
                    jj = g * 4 + q4
                    for hh in range(2):
                        nc.tensor.matmul(
                            out=sps[0:1, q4 * 128:(q4 + 1) * 128],
                            lhsT=ones_bf[:], rhs=xtr_all[:, j0 + jj, hh, :],
                            start=(hh == 0), stop=(hh == 1))
                        nc.tensor.matmul(
                            out=sps[0:1, 512 + q4 * 128:512 + (q4 + 1) * 128],
                            lhsT=ones_bf[:], rhs=ytr[:, jj, hh, :],
                            start=(hh == 0), stop=(hh == 1))
                t0 = g * 4 * 128
                nc.scalar.activation(out=mu_rows[0:1, t0:t0 + 512],
                                     in_=sps[0:1, 0:512],
                                     func=AF.Copy, bias=0.0, scale=1.0 / D)
                nc.scalar.activation(out=mus_row[0:1, t0:t0 + 512],
                                     in_=sps[0:1, 512:1024],
                                     func=AF.Copy, bias=0.0, scale=1.0 / D)

            # stats to partition layout via xbar of the mu_rows chunk
            nc.sync.dma_start(out=mu_rows[1:2, :], in_=mus_row[0:1, :])
            mu_part = small.tile([128, CHT, 16], BF, tag="mu_part")
            nc.sync.dma_start_transpose(mu_part[:], mu_rows[:, :])

            for inp, (ssq, invs, wslot) in enumerate(
                    ((ssq_r, invs_r, 1), (ssq_s, invs_s, 0))):
                mu = small.tile([128, CHT], F32, tag="mu_f")
                nc.vector.tensor_scalar(mu[:], mu_part[:, :, inp], 1.0, None,
                                        AL.mult)
                var = small.tile([128, CHT], F32, tag="var")
                nc.vector.scalar_tensor_tensor(
                    out=var[:], in0=mu[:], scalar=-1.0, op0=AL.mult,
                    op1=AL.mult, in1=mu[:])
                nc.vector.scalar_tensor_tensor(
                    out=var[:], in0=ssq[:, j0:j0 + CHT], scalar=1.0 / D,
                    op0=AL.mult, op1=AL.add, in1=var[:])
                sig = small.tile([128, CHT], F32, tag="sig")
                nc.scalar.activation(out=sig[:], in_=var[:], func=AF.Sqrt,
                                     bias=epsln[:, :], scale=1.0)
                nc.vector.reciprocal(out=invs[:, j0:j0 + CHT], in_=sig[:])
                nc.vector.scalar_tensor_tensor(
                    out=wcols[:, j0:j0 + CHT, wslot], in0=mu[:], scalar=-1.0,
                    op0=AL.mult, op1=AL.mult, in1=invs[:, j0:j0 + CHT])
                if inp == 0:
                    nc.vector.scalar_tensor_tensor(
                        out=arn[:, j0:j0 + CHT], in0=mu[:], scalar=-1.0,
                        op0=AL.mult, op1=AL.mult, in1=invs[:, j0:j0 + CHT])

            # q/k matmuls + evac + gram accumulation
            qt_c = qkp.tile([128, CHT, D], BF, tag="qt")
            kt_c = qkp.tile([128, CHT, D], BF, tag="kt")
            for jj in range(CHT):
                j = j0 + jj
                qkt = qkps.tile([128, 512], F32, tag="qk")
                qps = qkt[:, 0:256]
                kps = qkt[:, 256:512]
                for hh in range(2):
                    nc.tensor.matmul(out=qps, lhsT=ytr[:, jj, hh, :],
                                     rhs=wq_sb[:, hh, :],
                                     start=(hh == 0), stop=(hh == 1))
                    nc.tensor.matmul(out=kps, lhsT=xtr_all[:, j, hh, :],
                                     rhs=wk_sb[:, hh, :],
                                     start=(hh == 0), stop=(hh == 1))
                nc.vector.tensor_scalar(qt_c[:, jj, :], qps,
                                        invs_s[:, j:j + 1], None, AL.mult)
                nc.scalar.activation(out=kt_c[:, jj, :], in_=kps,
                                     func=AF.Copy, bias=0.0,
                                     scale=invs_r[:, j:j + 1])
                st = (j == 0)
                sp = (j == NT - 1)
                for hh in range(2):
                    qs = qt_c[:, jj, hh * 128:(hh + 1) * 128]
                    ks = kt_c[:, jj, hh * 128:(hh + 1) * 128]
                    nc.tensor.matmul(out=acc[:, hh * 128:(hh + 1) * 128],
                                     lhsT=qs, rhs=ks, start=st, stop=sp)
                    nc.tensor.matmul(
                        out=acc[:, 256 + hh * 128:256 + (hh + 1) * 128],
                        lhsT=qs, rhs=qs, start=st, stop=sp)
                    nc.tensor.matmul(
                        out=acc[:, 512 + hh * 128:512 + (hh + 1) * 128],
                        lhsT=ks, rhs=ks, start=st, stop=sp)
                wc = wcols[:, j, :]
                nc.tensor.matmul(out=acc[0:3, 768:1024], lhsT=wc,
                                 rhs=qt_c[:, jj, :], start=st, stop=sp)
                nc.tensor.matmul(out=acc[32:35, 768:1024], lhsT=wc,
                                 rhs=kt_c[:, jj, :], start=st, stop=sp)
                nc.tensor.matmul(out=acc[64:67, 768:771], lhsT=wc, rhs=wc,
                                 start=st, stop=sp)

    # ================= phase 2: pack + collective =================
    gt_sb = post.tile([128, 256], F32, tag="gt")
    nc.vector.tensor_scalar(gt_sb[:], acc[:, 0:256], 1.0, None, AL.mult)
    dq_sb = post.tile([128, H], F32, tag="dq")
    dk_sb = post.tile([128, H], F32, tag="dk")
    dscr = post.tile([128, 128], F32, tag="dscr")
    for hh in range(2):
        nc.vector.scalar_tensor_tensor(
            out=dscr[:], in0=acc[:, 256 + hh * 128:256 + (hh + 1) * 128],
            scalar=1.0, op0=AL.mult, op1=AL.mult, in1=eye_sb[:],
            accum_out=dq_sb[:, hh:hh + 1])
        nc.vector.scalar_tensor_tensor(
            out=dscr[:], in0=acc[:, 512 + hh * 128:512 + (hh + 1) * 128],
            scalar=1.0, op0=AL.mult, op1=AL.mult, in1=eye_sb[:],
            accum_out=dk_sb[:, hh:hh + 1])
    tq_sb = post.tile([3, 256], F32, tag="tq")
    tk_sb = post.tile([3, 256], F32, tag="tk")
    sc_sb = post.tile([3, 3], F32, tag="sc")
    nc.vector.tensor_scalar(tq_sb[:], acc[0:3, 768:1024], 1.0, None, AL.mult)
    nc.vector.tensor_scalar(tk_sb[:], acc[32:35, 768:1024], 1.0, None, AL.mult)
    nc.vector.tensor_scalar(sc_sb[:], acc[64:67, 768:771], 1.0, None, AL.mult)

    cc_in = dram.tile([128, PACKW], F32)
    cc_out = dram.tile([128, PACKW], F32)
    nc.gpsimd.dma_start(out=cc_in[:, 0:256], in_=gt_sb[:])
    nc.gpsimd.dma_start(out=cc_in[:, 256:258], in_=dq_sb[:])
    nc.gpsimd.dma_start(out=cc_in[:, 258:260], in_=dk_sb[:])
    nc.gpsimd.dma_start(out=cc_in[0:3, 260:516], in_=tq_sb[:])
    nc.gpsimd.dma_start(out=cc_in[0:3, 516:772], in_=tk_sb[:])
    nc.gpsimd.dma_start(out=cc_in[0:3, 772:775], in_=sc_sb[:])
    nc.gpsimd.collective_compute(
        "AllReduce", AL.add,
        replica_groups=[[0, 1], [2, 3], [4, 5], [6, 7]],
        ins=[cc_in.opt()], outs=[cc_out.opt()])

    with tc.tile_pool(name="mmps", bufs=2, space="PSUM") as mmps:
        # ---- v matmuls (no dependency on the collective -> overlaps it) ----
        for g in range(T // 512):
            vps = mmps.tile([128, 2, 512], F32, tag="mm")
            for hh in range(2):
                for dh in range(2):
                    nc.tensor.matmul(
                        out=vps[:, hh, :],
                        lhsT=wv_sb[:, dh, hh * 128:(hh + 1) * 128],
                        rhs=xtr_all[:, g * 4:(g + 1) * 4, dh, :],
                        start=(dh == 0), stop=(dh == 1))
            for hh in range(2):
                nc.scalar.activation(
                    out=vt_all[:, hh, g * 512:(g + 1) * 512],
                    in_=vps[:, hh, :], func=AF.Copy, bias=0.0, scale=1.0)

        # ================= phase 3: post-collective assembly ================
        red = post.tile([128, PACKW], F32, tag="red")
        nc.gpsimd.dma_start(out=red[:], in_=cc_out[:, :])
        rG = red[:, 0:256]
        rDq = red[:, 256:258]
        rDk = red[:, 258:260]

        # thin rows -> DRAM bounce; read back transposed / broadcast (f32)
        thin_d = dram.tile([6, 256], F32)
        nc.gpsimd.dma_start(out=thin_d[0:3, :], in_=red[0:3, 260:516])
        nc.gpsimd.dma_start(out=thin_d[3:6, :], in_=red[0:3, 516:772])
        sc_d = dram.tile([3, 3], F32)
        nc.gpsimd.dma_start(out=sc_d[:, :], in_=red[0:3, 772:775])

        # tcols[p, h, s] = thin row s at channel c=p of head h
        tcols = post.tile([128, H, 6], F32, tag="tcols")
        tap = thin_d[:, :]
        for hh in range(2):
            nc.sync.dma_start(out=tcols[:, hh, :], in_=bass.AP(
                tensor=tap.tensor, offset=tap.offset + hh * 128,
                ap=[[1, 128], [256, 6]]))
        # row broadcasts of RkA (row 3) and Rk0 (row 5)
        row3 = post.tile([128, 256], F32, tag="row3")
        row4 = post.tile([128, 256], F32, tag="row4")
        nc.sync.dma_start(out=row3[:], in_=_bcast(thin_d[3:4, :]))
        nc.sync.dma_start(out=row4[:], in_=_bcast(thin_d[5:6, :]))
        sAA = small.tile([128, 1], F32, tag="sAA")
        sAB = small.tile([128, 1], F32, tag="sAB")
        sA = small.tile([128, 1], F32, tag="sA")
        sBB = small.tile([128, 1], F32, tag="sBB")
        sB = small.tile([128, 1], F32, tag="sB")
        for dst, (r, c) in ((sAA, (0, 0)), (sAB, (0, 1)), (sA, (0, 2)),
                            (sBB, (1, 1)), (sB, (1, 2))):
            nc.sync.dma_start(out=dst[:], in_=_bcast(sc_d[r:r + 1, c:c + 1]))

        # row3 += s_k*Sab + bk2*Sa ; row4 += s_k*Sb + bk2*T
        t_r = post.tile([128, 256], F32, tag="t_r")
        nc.vector.scalar_tensor_tensor(out=t_r[:], in0=skr_b[:],
                                       scalar=sAB[:, :], op0=AL.mult,
                                       op1=AL.add, in1=row3[:])
        nc.vector.scalar_tensor_tensor(out=row3[:], in0=bkr_b[:],
                                       scalar=sA[:, :], op0=AL.mult,
                                       op1=AL.add, in1=t_r[:])
        nc.vector.scalar_tensor_tensor(out=t_r[:], in0=skr_b[:],
                                       scalar=sB[:, :], op0=AL.mult,
                                       op1=AL.add, in1=row4[:])
        nc.vector.scalar_tensor_tensor(out=row4[:], in0=bkr_b[:],
                                       scalar=float(N), op0=AL.mult,
                                       op1=AL.add, in1=t_r[:])

        # G assembly per head (in place on rG)
        for hh in range(2):
            Gh = rG[:, hh * 128:(hh + 1) * 128]
            nc.vector.scalar_tensor_tensor(
                out=Gh, in0=skr_b[:, hh * 128:(hh + 1) * 128],
                scalar=tcols[:, hh, 1:2], op0=AL.mult, op1=AL.add, in1=Gh)
            nc.vector.scalar_tensor_tensor(
                out=Gh, in0=bkr_b[:, hh * 128:(hh + 1) * 128],
                scalar=tcols[:, hh, 2:3], op0=AL.mult, op1=AL.add, in1=Gh)
            nc.vector.scalar_tensor_tensor(
                out=Gh, in0=row3[:, hh * 128:(hh + 1) * 128],
                scalar=sq_col[:, hh:hh + 1], op0=AL.mult, op1=AL.add, in1=Gh)
            nc.vector.scalar_tensor_tensor(
                out=Gh, in0=row4[:, hh * 128:(hh + 1) * 128],
                scalar=bq_col[:, hh:hh + 1], op0=AL.mult, op1=AL.add, in1=Gh)

        # norms
        def _norm2(dst, dvec, ucol, gcol, cA, c0, sXX, sX):
            t1 = small.tile([128, H], F32, tag="n_t1")
            nc.vector.tensor_tensor(out=t1[:], in0=ucol, in1=cA, op=AL.mult)
            nc.vector.scalar_tensor_tensor(out=dst[:], in0=t1[:], scalar=2.0,
                                           op0=AL.mult, op1=AL.add, in1=dvec)
            nc.vector.tensor_tensor(out=t1[:], in0=gcol, in1=c0, op=AL.mult)
            nc.vector.scalar_tensor_tensor(out=dst[:], in0=t1[:], scalar=2.0,
                                           op0=AL.mult, op1=AL.add, in1=dst[:])
            nc.vector.tensor_tensor(out=t1[:], in0=ucol, in1=ucol, op=AL.mult)
            nc.vector.scalar_tensor_tensor(out=dst[:], in0=t1[:],
                                           scalar=sXX[:, :], op0=AL.mult,
                                           op1=AL.add, in1=dst[:])
            nc.vector.tensor_tensor(out=t1[:], in0=ucol, in1=gcol, op=AL.mult)
            nc.vector.tensor_scalar(t1[:], t1[:], 2.0, None, AL.mult)
            nc.vector.scalar_tensor_tensor(out=dst[:], in0=t1[:],
                                           scalar=sX[:, :], op0=AL.mult,
                                           op1=AL.add, in1=dst[:])
            nc.vector.tensor_tensor(out=t1[:], in0=gcol, in1=gcol, op=AL.mult)
            nc.vector.scalar_tensor_tensor(out=dst[:], in0=t1[:],
                                           scalar=float(N), op0=AL.mult,
                                           op1=AL.add, in1=dst[:])

        qn2 = small.tile([128, H], F32, tag="qn2")
        kn2 = small.tile([128, H], F32, tag="kn2")
        _norm2(qn2, rDq, sq_col[:, :], bq_col[:, :], tcols[:, :, 0],
               tcols[:, :, 2], sAA, sA)
        _norm2(kn2, rDk, sk_col[:, :], bk_col[:, :], tcols[:, :, 4],
               tcols[:, :, 5], sBB, sB)

        def _invnorm(dst, src, mul_temp):
            sq = small.tile([128, H], F32, tag="invn_sq")
            nc.scalar.activation(out=sq[:], in_=src[:], func=AF.Sqrt,
                                 bias=zb[:, :], scale=1.0)
            nc.vector.tensor_scalar_max(sq[:], sq[:], EPS_NORM)
            nc.vector.reciprocal(out=dst[:], in_=sq[:])
            if mul_temp:
                nc.vector.tensor_tensor(out=dst[:], in0=dst[:],
                                        in1=temp_b[:, 0:H], op=AL.mult)

        invq = small.tile([128, H], F32, tag="invq")
        invk = small.tile([128, H], F32, tag="invk")
        _invnorm(invq, qn2, True)
        _invnorm(invk, kn2, False)

        # invk column -> per-head broadcast rows (via DRAM bounce).
        # Write transposed ([2, 128] row-contiguous) so the broadcast read
        # generates 512B-contiguous runs, not a 4-byte gather storm.
        ik_d = dram.tile([2, 128], F32)
        ik_ap = ik_d[:, :]
        nc.gpsimd.dma_start(out=bass.AP(
            tensor=ik_ap.tensor, offset=ik_ap.offset,
            ap=[[1, 128], [128, 2]]), in_=invk[:])
        ikb = post.tile([128, 2, 128], F32, tag="ikb")
        for hh in range(2):
            nc.sync.dma_start(out=ikb[:, hh, :], in_=_bcast(ik_d[hh:hh + 1, :]))

        # softmax per head
        attn = post.tile([128, 2, 128], F32, tag="attn")
        for hh in range(2):
            Gh = rG[:, hh * 128:(hh + 1) * 128]
            nc.vector.tensor_scalar(Gh, Gh, invq[:, hh:hh + 1], None, AL.mult)
            nc.vector.tensor_tensor(out=Gh, in0=Gh, in1=ikb[:, hh, :],
                                    op=AL.mult)
            rmax = small.tile([128, 1], F32, tag="rmax")
            nc.vector.tensor_reduce(out=rmax[:], in_=Gh, op=AL.max, axis=AX.X)
            nc.vector.tensor_scalar(rmax[:], rmax[:], -1.0, None, AL.mult)
            nc.scalar.activation(out=attn[:, hh, :], in_=Gh, func=AF.Exp,
                                 bias=rmax[:, :], scale=1.0)
            rsum = small.tile([128, 1], F32, tag="rsum")
            nc.vector.tensor_reduce(out=rsum[:], in_=attn[:, hh, :], op=AL.add,
                                    axis=AX.X)
            nc.vector.reciprocal(out=rsum[:], in_=rsum[:])
            nc.vector.tensor_scalar(attn[:, hh, :], attn[:, hh, :],
                                    rsum[:, :], None, AL.mult)

        # attn^T via PE -> bf16
        attnT = post.tile([128, 2, 128], BF, tag="attnT")
        for hh in range(2):
            tp = mmps.tile([128, 2, 512], F32, tag="mm")
            nc.tensor.transpose(tp[:, 0, 0:128], attn[:, hh, :], eye_sb[:])
            nc.scalar.activation(out=attnT[:, hh, :], in_=tp[:, 0, 0:128],
                                 func=AF.Copy, bias=0.0, scale=1.0)

        # E vectors ([c, head, (E1,E0)]) and F rows
        e_sb = post.tile([128, 2, 2], BF, tag="e_sb")
        for hh in range(2):
            eps_mm = mmps.tile([128, 2, 512], F32, tag="mm")
            svbv = small.tile([128, 2], BF, tag="svbv")
            nc.vector.tensor_scalar(svbv[:, 0:1], sv_col[:, hh, :], 1.0, None,
                                    AL.mult)
            nc.vector.tensor_scalar(svbv[:, 1:2], bv_col[:, hh, :], 1.0, None,
                                    AL.mult)
            nc.tensor.matmul(out=eps_mm[:, 0, 0:2], lhsT=attnT[:, hh, :],
                             rhs=svbv[:], start=True, stop=True)
            nc.vector.tensor_scalar(e_sb[:, hh, :], eps_mm[:, 0, 0:2], 1.0,
                                    None, AL.mult)
        fps = mmps.tile([128, 2, 512], F32, tag="mm")
        for hh in range(2):
            nc.tensor.matmul(out=fps[0:1, 0, 0:256], lhsT=e_sb[:, hh, 0:1],
                             rhs=wo_sb[:, hh, :], start=(hh == 0),
                             stop=(hh == 1))
            nc.tensor.matmul(out=fps[32:33, 0, 0:256], lhsT=e_sb[:, hh, 1:2],
                             rhs=wo_sb[:, hh, :], start=(hh == 0),
                             stop=(hh == 1))
        f1_sb = post.tile([1, 256], F32, tag="f1_sb")
        f2_sb = post.tile([1, 256], F32, tag="f2_sb")
        nc.vector.tensor_scalar(f1_sb[:, :], fps[0:1, 0, 0:256], 1.0, None,
                                AL.mult)
        nc.vector.tensor_scalar(f2_sb[:, :], fps[32:33, 0, 0:256], 1.0, None,
                                AL.mult)
        nc.vector.tensor_tensor(out=f2_sb[:, :], in0=f2_sb[:, :],
                                in1=bo_row[:, :], op=AL.add)
        f_d = dram.tile([2, 256], F32)
        nc.gpsimd.dma_start(out=f_d[0:1, :], in_=f1_sb[:, :])
        nc.gpsimd.dma_start(out=f_d[1:2, :], in_=f2_sb[:, :])
        f1b = post.tile([128, 256], F32, tag="f1b")
        f2b = post.tile([128, 256], F32, tag="f2b")
        nc.sync.dma_start(out=f1b[:], in_=_bcast(f_d[0:1, :]))
        nc.sync.dma_start(out=f2b[:], in_=_bcast(f_d[1:2, :]))

        # ================= phase 4: av + wo + final evac =================
        for g in range(T // 512):
            avps = mmps.tile([128, 2, 512], F32, tag="mm")
            for hh in range(2):
                nc.tensor.matmul(out=avps[:, hh, :], lhsT=attnT[:, hh, :],
                                 rhs=vt_all[:, hh, g * 512:(g + 1) * 512],
                                 start=True, stop=True)
            av_sb = outp.tile([128, 2, 512], BF, tag="av_sb")
            for hh in range(2):
                nc.scalar.activation(out=av_sb[:, hh, :], in_=avps[:, hh, :],
                                     func=AF.Copy, bias=0.0, scale=1.0)
            ops = mmps.tile([128, 2, 512], F32, tag="mm")
            out_sb = outp.tile([128, 4, 256], F32, tag="out_sb")
            for q4 in range(4):
                j = g * 4 + q4
                for hh in range(2):
                    nc.tensor.matmul(
                        out=ops[:, q4 // 2, (q4 % 2) * 256:(q4 % 2 + 1) * 256],
                        lhsT=av_sb[:, hh, q4 * 128:(q4 + 1) * 128],
                        rhs=wo_sb[:, hh, :], start=(hh == 0), stop=(hh == 1))
                t1 = outp.tile([128, 256], F32, tag="t1")
                nc.vector.scalar_tensor_tensor(
                    out=t1[:], in0=f1b[:], scalar=arn[:, j:j + 1],
                    op0=AL.mult, op1=AL.add, in1=f2b[:])
                nc.vector.scalar_tensor_tensor(
                    out=out_sb[:, q4, :],
                    in0=ops[:, q4 // 2, (q4 % 2) * 256:(q4 % 2 + 1) * 256],
                    scalar=invs_r[:, j:j + 1], op0=AL.mult, op1=AL.add,
                    in1=t1[:])
            nc.sync.dma_start(out=outv[:, g * 4:(g + 1) * 4, :], in_=out_sb[:])


# ======================= host side =======================

def _prep_shared(inputs):
    f32 = np.float32
    Wq = np.asarray(inputs["Wq"], f32)
    bq = np.asarray(inputs["bq"], f32)
    Wkv = np.asarray(inputs["Wkv"], f32)
    bkv = np.asarray(inputs["bkv"], f32)
    Wo = np.asarray(inputs["Wo"], f32)
    bo = np.asarray(inputs["bo"], f32)
    lnS_w = np.asarray(inputs["lnS_w"], f32)
    lnS_b = np.asarray(inputs["lnS_b"], f32)
    lnR_w = np.asarray(inputs["lnR_w"], f32)
    lnR_b = np.asarray(inputs["lnR_b"], f32)
    temp = np.asarray(inputs["temperature"], f32).reshape(H)

    Wk, Wv = Wkv[:D], Wkv[D:]
    Wqp = Wq * lnS_w[None, :]
    Wkp = Wk * lnR_w[None, :]
    Wvp = Wv * lnR_w[None, :]
    bq2 = Wq @ lnS_b + bq
    bk2 = Wk @ lnR_b + bkv[:D]
    bv2 = Wv @ lnR_b + bkv[D:]
    s_q, s_k, s_v = Wqp.sum(1), Wkp.sum(1), Wvp.sum(1)

    def colh(v):
        return np.ascontiguousarray(v.reshape(H, 128).T, f32)

    return {
        "wqT": np.ascontiguousarray(Wqp.T).astype(BF16),
        "wkT": np.ascontiguousarray(Wkp.T).astype(BF16),
        "wvT": np.ascontiguousarray(Wvp.T).astype(BF16),
        "woT": np.ascontiguousarray(Wo.T).astype(BF16),
        "sv_col": s_v.reshape(D, 1).astype(BF16),
        "bv2_col": bv2.reshape(D, 1).astype(BF16),
        "sq_col": colh(s_q),
        "bq2_col": colh(bq2),
        "sk_col": colh(s_k),
        "bk2_col": colh(bk2),
        "sk_row": s_k.reshape(1, D).astype(f32),
        "bk2_row": bk2.reshape(1, D).astype(f32),
        "bo_row": bo.reshape(1, D).astype(f32),
        "eye": np.eye(128, dtype=f32),
        "temp": temp.reshape(1, H).astype(f32),
    }


def _get_nc():
    if "nc" not in _nc_cache:
        _nc_cache["nc"] = _build_nc()
    return _nc_cache["nc"]


def run(inputs, trace=False):
    nc = _get_nc()
    shared = _prep_shared(inputs)
    iR = np.asarray(inputs["input_R"], np.float32)
    iS = np.asarray(inputs["input_S"], np.float32)
    in_maps = []
    for ci in range(N_CORES):
        b, half = ci // 2, ci % 2
        m = dict(shared)
        m["x_r"] = np.ascontiguousarray(iR[b, half * T:(half + 1) * T])
        m["x_s"] = np.ascontiguousarray(iS[b, half * T:(half + 1) * T])
        in_maps.append(m)
    res = run_bass_kernel_spmd(nc, in_maps, list(range(N_CORES)), trace=trace)
    out = np.zeros((B, N, D), np.float32)
    for ci in range(N_CORES):
        b, half = ci // 2, ci % 2
        out[b, half * T:(half + 1) * T] = res.results[ci]["out"]
    return out, res


def kernel(**inputs):
    out, _ = run(inputs, trace=False)
    return out



# revision 7
# speedup vs baseline: 1.0261x; 1.0261x over previous
"""Channel-attention (XCA-style) Trainium2 kernel, 8-way SPMD — v2.

Shapes (hardcoded): B=4, N=16384, D=256, H=2 heads, c=128.
Sharding: core ci -> batch b=ci//2, token half ci%2 (T=8192 tokens/core).

v2 strategy (C-matrix factorization): instead of materializing q/k per
token, accumulate token-contraction Gram matrices of the *scaled raw*
inputs:  C_sr = sum_t (u_t s_t)(w_t r_t)^T,  C_ss, C_rr, plus moment
vectors against [1, a_t, c_t].  The attention logits G, and the q/k L2
norms, are then small [256x256] products of C with the LN-folded weight
matrices, assembled per core and pair-AllReduced ([128,260] f32 only).
The entire v/attn@v/Wo output path collapses to one matrix
Pp = Wo_blk . attn_blk . WvR applied per token:  out_t = Pp r'_t + c_t f1 + f2,
computed from the d-major transpose of the scaled kv input (saved during
the streaming pass).  No q/k/v projections, no q/k evacuations, no
mean matmuls, one DMA transpose instead of two.
"""
import sys, types

sys.path.insert(0, "/opt/trn_rl_repo")

try:
    import antenv
    if "antenv.axon_hooks" not in sys.modules:
        _hooks = types.ModuleType("antenv.axon_hooks")
        _hooks._hook = None
        _hooks.set_axon_ntff_profile_hook = lambda h: setattr(_hooks, "_hook", h)
        _hooks.get_axon_ntff_profile_hook = lambda: _hooks._hook
        sys.modules["antenv.axon_hooks"] = _hooks
        antenv.axon_hooks = _hooks
        from trn_agent_boot.trn_boot import _ntff_profile_via_ctypes
        _hooks.set_axon_ntff_profile_hook(
            _ntff_profile_via_ctypes("/opt/axon/libaxon_pjrt.so"))
except Exception:
    pass

import numpy as np
import ml_dtypes

import concourse.bass as bass
import concourse.bacc as bacc
import concourse.mybir as mybir
import concourse.tile as tile
from concourse.bass_utils import run_bass_kernel_spmd

BF16 = ml_dtypes.bfloat16
F32 = mybir.dt.float32
BF = mybir.dt.bfloat16
AL = mybir.AluOpType
AF = mybir.ActivationFunctionType
AX = mybir.AxisListType

B, N, D, H = 4, 16384, 256, 2
T = N // 2                  # tokens per core
NT = 64                     # token tiles per core (inner j), token = p*64 + j
CHT = 8                     # tiles per chunk
NCH = NT // CHT             # 8 chunks
EPS_LN = 1e-5
EPS_NORM = 1e-12
N_CORES = 8
TCORE = float(T)

# combo tile column layout (bf16): [pad 0:13 | wcol 13:16 | r' 16:272 | s' 272:528]
WC0 = 13
RP0 = 16
SP0 = 272
CW = 528

_nc_cache = {}


def _bcast(ap, rows=128):
    return bass.AP(tensor=ap.tensor, offset=ap.offset,
                   ap=[[0, rows]] + [list(x) for x in ap.ap[1:]])


def _build_nc():
    nc = bacc.Bacc("TRN2", target_bir_lowering=False, debug=False,
                   num_devices=N_CORES)

    def ein(name, shape, dt=F32):
        return nc.dram_tensor(name, list(shape), dt, kind="ExternalInput")

    d_s = ein("x_s", [T, D])            # q source shard (input_S)
    d_r = ein("x_r", [T, D])            # kv source shard (input_R)
    d_wqsT = ein("wqsT", [128, 2, D], BF)
    d_wkrT = ein("wkrT", [128, 2, D], BF)
    d_wqs = ein("wqs", [128, 2, D], BF)
    d_wkr = ein("wkr", [128, 2, D], BF)
    d_wvr = ein("wvr", [128, 2, D], BF)
    d_woT = ein("woT", [128, 2, D], BF)
    d_svbv = ein("svbv", [128, 2, 2], BF)
    d_sqc = ein("sq_col", [128, 2])
    d_bq2c = ein("bq2_col", [128, 2])
    d_skc = ein("sk_col", [128, 2])
    d_bk2c = ein("bk2_col", [128, 2])
    d_boc = ein("bo_col", [128, 2])
    d_skr = ein("sk_row", [1, D])
    d_bk2r = ein("bk2_row", [1, D])
    d_eye = ein("eye", [128, 128], BF)
    d_temp = ein("temp", [1, 2])
    d_out = nc.dram_tensor("out", [T, D], BF, kind="ExternalOutput")

    sv = d_s.rearrange("(p j) d -> p j d", p=128)
    rv = d_r.rearrange("(p j) d -> p j d", p=128)
    outv = d_out.rearrange("(p j) d -> p j d", p=128)

    with tile.TileContext(nc) as tc:
        import contextlib
        with contextlib.ExitStack() as ctx:
            _body(ctx, tc, nc, sv, rv, outv, d_wqsT, d_wkrT, d_wqs, d_wkr,
                  d_wvr, d_woT, d_svbv, d_sqc, d_bq2c, d_skc, d_bk2c, d_boc,
                  d_skr, d_bk2r, d_eye, d_temp)
    nc.finalize()
    return nc


def _body(ctx, tc, nc, sv, rv, outv, d_wqsT, d_wkrT, d_wqs, d_wkr, d_wvr,
          d_woT, d_svbv, d_sqc, d_bq2c, d_skc, d_bk2c, d_boc, d_skr, d_bk2r,
          d_eye, d_temp):
    E = ctx.enter_context
    consts = E(tc.tile_pool(name="consts", bufs=1))
    stats = E(tc.tile_pool(name="stats", bufs=1))
    ldp = E(tc.tile_pool(name="ldp", bufs=2))
    cbp = E(tc.tile_pool(name="cbp", bufs=2))
    small = E(tc.tile_pool(name="small", bufs=4))
    pers = E(tc.tile_pool(name="pers", bufs=1))
    post = E(tc.tile_pool(name="post", bufs=1))
    outp = E(tc.tile_pool(name="outp", bufs=2))
    dram = E(tc.tile_pool(name="dram", bufs=1, space="DRAM"))

    # ---------------- constants ----------------
    wqsT = consts.tile([128, 2, D], BF, tag="wqsT")
    wkrT = consts.tile([128, 2, D], BF, tag="wkrT")
    wqs = consts.tile([128, 2, D], BF, tag="wqs")
    wkr = consts.tile([128, 2, D], BF, tag="wkr")
    wvr = consts.tile([128, 2, D], BF, tag="wvr")
    woT = consts.tile([128, 2, D], BF, tag="woT")
    for dst, src in ((wqsT, d_wqsT), (wkrT, d_wkrT), (wqs, d_wqs),
                     (wkr, d_wkr), (wvr, d_wvr), (woT, d_woT)):
        nc.sync.dma_start(out=dst[:], in_=src[:, :, :])
    svbv = consts.tile([128, 2, 2], BF, tag="svbv")
    nc.sync.dma_start(out=svbv[:], in_=d_svbv[:, :, :])
    sq_col = consts.tile([128, 2], F32, tag="sqc")
    bq2_col = consts.tile([128, 2], F32, tag="bq2c")
    sk_col = consts.tile([128, 2], F32, tag="skc")
    bk2_col = consts.tile([128, 2], F32, tag="bk2c")
    bo_col = consts.tile([128, 2], F32, tag="boc")
    for dst, src in ((sq_col, d_sqc), (bq2_col, d_bq2c), (sk_col, d_skc),
                     (bk2_col, d_bk2c), (bo_col, d_boc)):
        nc.sync.dma_start(out=dst[:], in_=src[:, :])
    skrow_b = consts.tile([128, D], F32, tag="skrb")
    bk2row_b = consts.tile([128, D], F32, tag="bk2rb")
    nc.sync.dma_start(out=skrow_b[:], in_=_bcast(d_skr[:, :]))
    nc.sync.dma_start(out=bk2row_b[:], in_=_bcast(d_bk2r[:, :]))
    eye_sb = consts.tile([128, 128], BF, tag="eye")
    nc.sync.dma_start(out=eye_sb[:], in_=d_eye[:, :])
    temp_b = consts.tile([128, 2], F32, tag="tempb")
    nc.sync.dma_start(out=temp_b[:], in_=_bcast(d_temp[:, :]))
    ones_row = consts.tile([1, 128], BF, tag="ones")
    nc.vector.memset(ones_row[:], 1.0)
    epsln = consts.tile([128, 1], F32, tag="epsln")
    nc.vector.memset(epsln[:], EPS_LN)
    zb = consts.tile([128, 1], F32, tag="zb")
    nc.vector.memset(zb[:], 0.0)

    # ---------------- state ----------------
    ssq_s = stats.tile([128, NT], F32, tag="ssq_s")
    ssq_r = stats.tile([128, NT], F32, tag="ssq_r")
    c_col = stats.tile([128, NT], F32, tag="c_col")
    sqscr = stats.tile([128, 256], BF, tag="sqscr")   # ACT square scratch
    sqscr2 = stats.tile([128, 256], BF, tag="sqscr2")  # DVE square scratch
    rT_all = pers.tile([128, NT, 2, 128], BF, tag="rT")

    # ================= phase A: stream chunks =================
    with tc.tile_pool(name="accA", bufs=1, space="PSUM") as accA:
        b_sr0 = accA.tile([128, 259], F32, tag="b_sr0")
        b_sr1 = accA.tile([128, 259], F32, tag="b_sr1")
        b_rr0 = accA.tile([128, 259], F32, tag="b_rr0")
        b_rr1 = accA.tile([128, 259], F32, tag="b_rr1")
        b_ss0 = accA.tile([128, 256], F32, tag="b_ss0")
        b_ss1 = accA.tile([128, 256], F32, tag="b_ss1")
        b_wg = accA.tile([128, 3], F32, tag="b_wg")

        for ch in range(NCH):
            j0 = ch * CHT
            s_raw = ldp.tile([128, CHT, D], BF, tag="s_raw")
            r_raw = ldp.tile([128, CHT, D], BF, tag="r_raw")
            nc.gpsimd.dma_start(out=s_raw[:], in_=sv[:, j0:j0 + CHT, :])
            nc.gpsimd.dma_start(out=r_raw[:], in_=rv[:, j0:j0 + CHT, :])
            combo = cbp.tile([128, CHT, CW], BF, tag="combo")

            # --- per-token LN stats ---
            sums_s = small.tile([128, CHT], F32, tag="sums_s")
            sums_r = small.tile([128, CHT], F32, tag="sums_r")
            nc.vector.tensor_reduce(out=sums_s[:], in_=s_raw[:], axis=AX.X,
                                    op=AL.add)
            nc.vector.tensor_reduce(out=sums_r[:], in_=r_raw[:], axis=AX.X,
                                    op=AL.add)
            for jj in range(CHT):
                j = j0 + jj
                nc.scalar.activation(out=sqscr[:], in_=s_raw[:, jj, :],
                                     func=AF.Square,
                                     accum_out=ssq_s[:, j:j + 1])
                if jj % 2 == 0:
                    nc.scalar.activation(out=sqscr[:], in_=r_raw[:, jj, :],
                                         func=AF.Square,
                                         accum_out=ssq_r[:, j:j + 1])
                else:
                    nc.vector.scalar_tensor_tensor(
                        out=sqscr2[:], in0=r_raw[:, jj, :], scalar=0.0,
                        op0=AL.bypass, op1=AL.mult, in1=r_raw[:, jj, :],
                        accum_out=ssq_r[:, j:j + 1])

            for inp in range(2):
                sums = sums_s if inp == 0 else sums_r
                ssq = ssq_s if inp == 0 else ssq_r
                mu = small.tile([128, CHT], F32, tag="mu")
                nc.vector.tensor_scalar(mu[:], sums[:], 1.0 / D, None, AL.mult)
                musq = small.tile([128, CHT], F32, tag="musq")
                nc.vector.tensor_tensor(out=musq[:], in0=mu[:], in1=mu[:],
                                        op=AL.mult)
                var = small.tile([128, CHT], F32, tag="var")
                nc.vector.scalar_tensor_tensor(
                    out=var[:], in0=ssq[:, j0:j0 + CHT], scalar=1.0 / D,
                    op0=AL.mult, op1=AL.subtract, in1=musq[:])
                sig = small.tile([128, CHT], F32, tag="sig")
                nc.scalar.activation(out=sig[:], in_=var[:], func=AF.Sqrt,
                                     bias=epsln[:, :], scale=1.0)
                invs = small.tile([128, CHT], F32, tag="invs")
                nc.vector.reciprocal(out=invs[:], in_=sig[:])
                if inp == 0:
                    # a = -mu_s * u  -> wcol col 14 (bf16)
                    nc.vector.scalar_tensor_tensor(
                        out=combo[:, :, WC0 + 1], in0=mu[:], scalar=-1.0,
                        op0=AL.mult, op1=AL.mult, in1=invs[:])
                    # s' = u * s  (DVE, per tile)
                    for jj in range(CHT):
                        nc.vector.tensor_scalar(
                            combo[:, jj, SP0:SP0 + 256], s_raw[:, jj, :],
                            invs[:, jj:jj + 1], None, AL.mult)
                else:
                    # c = -mu_r * w  -> persistent f32 + wcol col 15
                    nc.vector.scalar_tensor_tensor(
                        out=c_col[:, j0:j0 + CHT], in0=mu[:], scalar=-1.0,
                        op0=AL.mult, op1=AL.mult, in1=invs[:])
                    nc.gpsimd.tensor_scalar(combo[:, :, WC0 + 2],
                                            c_col[:, j0:j0 + CHT], 1.0, None,
                                            AL.mult)
                    # r' = w * r  (ACT, per tile)
                    for jj in range(CHT):
                        nc.scalar.activation(
                            out=combo[:, jj, RP0:RP0 + 256],
                            in_=r_raw[:, jj, :], func=AF.Copy, bias=0.0,
                            scale=invs[:, jj:jj + 1])
            nc.gpsimd.memset(combo[:, :, WC0], 1.0)

            # --- accumulation matmuls + transpose ---
            for jj in range(CHT):
                j = j0 + jj
                st = (j == 0)
                sp = (j == NT - 1)
                rhs_wr = combo[:, jj, WC0:RP0 + 256]     # [wcol | r'] 259
                rhs_ss = combo[:, jj, SP0:SP0 + 256]     # s' 256
                for h in range(2):
                    lh_s = combo[:, jj, SP0 + h * 128:SP0 + (h + 1) * 128]
                    nc.tensor.matmul(out=(b_sr0 if h == 0 else b_sr1)[:],
                                     lhsT=lh_s, rhs=rhs_wr, start=st, stop=sp)
                    nc.tensor.matmul(out=(b_ss0 if h == 0 else b_ss1)[:],
                                     lhsT=lh_s, rhs=rhs_ss, start=st, stop=sp)
                for h in range(2):
                    lh_r = combo[:, jj, RP0 + h * 128:RP0 + (h + 1) * 128]
                    nc.tensor.matmul(out=(b_rr0 if h == 0 else b_rr1)[:],
                                     lhsT=lh_r, rhs=rhs_wr, start=st, stop=sp)
                nc.tensor.matmul(out=b_wg[0:3, :],
                                 lhsT=combo[:, jj, WC0:WC0 + 3],
                                 rhs=combo[:, jj, WC0:WC0 + 3],
                                 start=st, stop=sp)
                nc.sync.dma_start_transpose(rT_all[:, j, :, :],
                                            combo[:, jj, RP0:RP0 + 256])

        # ================= phase B: evac + small matmuls =================
        csr_sb = post.tile([128, 2, 259], BF, tag="csr")
        crr_sb = post.tile([128, 2, 259], BF, tag="crr")
        css_sb = post.tile([128, 2, 256], BF, tag="css")
        sg_sb = post.tile([3, 3], F32, tag="sg")
        nc.vector.tensor_scalar(csr_sb[:, 0, :], b_sr0[:], 1.0, None, AL.mult)
        nc.vector.tensor_scalar(csr_sb[:, 1, :], b_sr1[:], 1.0, None, AL.mult)
        nc.scalar.activation(out=crr_sb[:, 0, :], in_=b_rr0[:], func=AF.Copy)
        nc.scalar.activation(out=crr_sb[:, 1, :], in_=b_rr1[:], func=AF.Copy)
        nc.vector.tensor_scalar(css_sb[:, 0, :], b_ss0[:], 1.0, None, AL.mult)
        nc.vector.tensor_scalar(css_sb[:, 1, :], b_ss1[:], 1.0, None, AL.mult)
        nc.vector.tensor_scalar(sg_sb[:], b_wg[0:3, :], 1.0, None, AL.mult)

    # S-gram scalars -> per-partition broadcasts via DRAM bounce
    sg_d = dram.tile([3, 3], F32)
    nc.gpsimd.dma_start(out=sg_d[:, :], in_=sg_sb[:])
    sSa = small.tile([128, 1], F32, tag="sSa")
    sSc = small.tile([128, 1], F32, tag="sSc")
    sSaa = small.tile([128, 1], F32, tag="sSaa")
    sSac = small.tile([128, 1], F32, tag="sSac")
    sScc = small.tile([128, 1], F32, tag="sScc")
    for dst, (rr, cc2) in ((sSa, (0, 1)), (sSc, (0, 2)), (sSaa, (1, 1)),
                           (sSac, (1, 2)), (sScc, (2, 2))):
        nc.sync.dma_start(out=dst[:], in_=_bcast(sg_d[rr:rr + 1, cc2:cc2 + 1]))

    with tc.tile_pool(name="pb", bufs=1, space="PSUM") as pb:
        thq = pb.tile([128, 2, 3], F32, tag="thq")   # [beta|eps|alpha] per head
        thk = pb.tile([128, 2, 3], F32, tag="thk")   # [delta|gam|zeta]
        xh_ps = pb.tile([128, 2, 256], F32, tag="xh")
        g_ps = pb.tile([128, 2, 128], F32, tag="g")
        z_ps = pb.tile([128, 2, 256], F32, tag="z")
        tr_ps = pb.tile([128, 2, 128], BF, tag="tr")

        for ih in range(2):
            for lh in range(2):
                nc.tensor.matmul(out=thq[:, ih, :],
                                 lhsT=wqsT[:, lh, ih * 128:(ih + 1) * 128],
                                 rhs=csr_sb[:, lh, 0:3],
                                 start=(lh == 0), stop=(lh == 1))
                nc.tensor.matmul(out=thk[:, ih, :],
                                 lhsT=wkrT[:, lh, ih * 128:(ih + 1) * 128],
                                 rhs=crr_sb[:, lh, 0:3],
                                 start=(lh == 0), stop=(lh == 1))
        # X_h = Wq_h C_sr   [i in h, j(256)]
        for h in range(2):
            for lh in range(2):
                nc.tensor.matmul(out=xh_ps[:, h, :],
                                 lhsT=wqsT[:, lh, h * 128:(h + 1) * 128],
                                 rhs=csr_sb[:, lh, 3:259],
                                 start=(lh == 0), stop=(lh == 1))
        x_sb = post.tile([128, 2, 256], BF, tag="x_sb")
        nc.vector.tensor_scalar(x_sb[:, 0, :], xh_ps[:, 0, :], 1.0, None,
                                AL.mult)
        nc.vector.tensor_scalar(x_sb[:, 1, :], xh_ps[:, 1, :], 1.0, None,
                                AL.mult)
        xT_sb = post.tile([128, 2, 2, 128], BF, tag="xT")
        for h in range(2):
            for jh in range(2):
                nc.tensor.transpose(tr_ps[:, jh, :],
                                    x_sb[:, h, jh * 128:(jh + 1) * 128],
                                    eye_sb[:])
            for jh in range(2):
                nc.scalar.activation(out=xT_sb[:, h, jh, :],
                                     in_=tr_ps[:, jh, :], func=AF.Copy)
        # G_h = X_h^T-contract with WkR
        for h in range(2):
            for jh in range(2):
                nc.tensor.matmul(out=g_ps[:, h, :],
                                 lhsT=xT_sb[:, h, jh, :],
                                 rhs=wkrT[:, jh, h * 128:(h + 1) * 128],
                                 start=(jh == 0), stop=(jh == 1))
        # dq = diag(WqS C_ss WqS^T), dk likewise
        dq_sb = small.tile([128, 2], F32, tag="dq")
        dk_sb = small.tile([128, 2], F32, tag="dk")
        dscr = post.tile([128, 256], F32, tag="dscr")
        for a in range(2):
            for lh in range(2):
                nc.tensor.matmul(out=z_ps[:, a, :],
                                 lhsT=wqsT[:, lh, a * 128:(a + 1) * 128],
                                 rhs=css_sb[:, lh, :],
                                 start=(lh == 0), stop=(lh == 1))
        for a in range(2):
            nc.vector.scalar_tensor_tensor(
                out=dscr[:], in0=z_ps[:, a, :], scalar=0.0, op0=AL.bypass,
                op1=AL.mult, in1=wqs[:, a, :], accum_out=dq_sb[:, a:a + 1])
        for a in range(2):
            for lh in range(2):
                nc.tensor.matmul(out=z_ps[:, a, :],
                                 lhsT=wkrT[:, lh, a * 128:(a + 1) * 128],
                                 rhs=crr_sb[:, lh, 3:259],
                                 start=(lh == 0), stop=(lh == 1))
        for a in range(2):
            nc.vector.scalar_tensor_tensor(
                out=dscr[:], in0=z_ps[:, a, :], scalar=0.0, op0=AL.bypass,
                op1=AL.mult, in1=wkr[:, a, :], accum_out=dk_sb[:, a:a + 1])

        # norms: nq2 = dq + 2 eps*sq + 2 beta*bq2 + Saa sq^2 + 2 Sa sq bq2 + T bq2^2
        nq2 = small.tile([128, 2], F32, tag="nq2")
        nk2 = small.tile([128, 2], F32, tag="nk2")
        t1 = small.tile([128, 2], F32, tag="t1")
        for (dst, d_sb, th, ucol, gcol, sXX, sX) in (
                (nq2, dq_sb, thq, sq_col, bq2_col, sSaa, sSa),
                (nk2, dk_sb, thk, sk_col, bk2_col, sScc, sSc)):
            nc.vector.tensor_tensor(out=t1[:], in0=th[:, :, 1], in1=ucol[:],
                                    op=AL.mult)
            nc.vector.scalar_tensor_tensor(out=dst[:], in0=t1[:], scalar=2.0,
                                           op0=AL.mult, op1=AL.add,
                                           in1=d_sb[:])
            nc.vector.tensor_tensor(out=t1[:], in0=th[:, :, 0], in1=gcol[:],
                                    op=AL.mult)
            nc.vector.scalar_tensor_tensor(out=dst[:], in0=t1[:], scalar=2.0,
                                           op0=AL.mult, op1=AL.add, in1=dst[:])
            nc.vector.tensor_tensor(out=t1[:], in0=ucol[:], in1=ucol[:],
                                    op=AL.mult)
            nc.vector.scalar_tensor_tensor(out=dst[:], in0=t1[:],
                                           scalar=sXX[:, :], op0=AL.mult,
                                           op1=AL.add, in1=dst[:])
            nc.vector.tensor_tensor(out=t1[:], in0=ucol[:], in1=gcol[:],
                                    op=AL.mult)
            nc.vector.tensor_scalar(t1[:], t1[:], 2.0, None, AL.mult)
            nc.vector.scalar_tensor_tensor(out=dst[:], in0=t1[:],
                                           scalar=sX[:, :], op0=AL.mult,
                                           op1=AL.add, in1=dst[:])
            nc.vector.tensor_tensor(out=t1[:], in0=gcol[:], in1=gcol[:],
                                    op=AL.mult)
            nc.vector.scalar_tensor_tensor(out=dst[:], in0=t1[:],
                                           scalar=TCORE, op0=AL.mult,
                                           op1=AL.add, in1=dst[:])

        # G rank-1 corrections: rows for sq/bq2 terms via DRAM bounce
        r1c = small.tile([128, 2], F32, tag="r1c")
        r2c = small.tile([128, 2], F32, tag="r2c")
        nc.vector.scalar_tensor_tensor(out=r1c[:], in0=sk_col[:],
                                       scalar=sSac[:, :], op0=AL.mult,
                                       op1=AL.add, in1=thk[:, :, 1])
        nc.vector.scalar_tensor_tensor(out=r1c[:], in0=bk2_col[:],
                                       scalar=sSa[:, :], op0=AL.mult,
                                       op1=AL.add, in1=r1c[:])
        nc.vector.scalar_tensor_tensor(out=r2c[:], in0=sk_col[:],
                                       scalar=sSc[:, :], op0=AL.mult,
                                       op1=AL.add, in1=thk[:, :, 0])
        nc.vector.scalar_tensor_tensor(out=r2c[:], in0=bk2_col[:],
                                       scalar=TCORE, op0=AL.mult,
                                       op1=AL.add, in1=r2c[:])
        r1_d = dram.tile([1, 256], F32)
        r2_d = dram.tile([1, 256], F32)
        for dcol, dd in ((r1c, r1_d), (r2c, r2_d)):
            ap = dd[:, :]
            nc.gpsimd.dma_start(out=bass.AP(tensor=ap.tensor, offset=ap.offset,
                                            ap=[[1, 128], [128, 2]]),
                                in_=dcol[:])
        r1_b = post.tile([128, 256], F32, tag="r1b")
        r2_b = post.tile([128, 256], F32, tag="r2b")
        nc.sync.dma_start(out=r1_b[:], in_=_bcast(r1_d[0:1, :]))
        nc.sync.dma_start(out=r2_b[:], in_=_bcast(r2_d[0:1, :]))
        for h in range(2):
            gh = g_ps[:, h, :]
            nc.vector.scalar_tensor_tensor(
                out=gh, in0=skrow_b[:, h * 128:(h + 1) * 128],
                scalar=thq[:, h, 2:3], op0=AL.mult, op1=AL.add, in1=gh)
            nc.vector.scalar_tensor_tensor(
                out=gh, in0=bk2row_b[:, h * 128:(h + 1) * 128],
                scalar=thq[:, h, 0:1], op0=AL.mult, op1=AL.add, in1=gh)
            nc.vector.scalar_tensor_tensor(
                out=gh, in0=r1_b[:, h * 128:(h + 1) * 128],
                scalar=sq_col[:, h:h + 1], op0=AL.mult, op1=AL.add, in1=gh)
            nc.vector.scalar_tensor_tensor(
                out=gh, in0=r2_b[:, h * 128:(h + 1) * 128],
                scalar=bq2_col[:, h:h + 1], op0=AL.mult, op1=AL.add, in1=gh)

        # pack [G0 | G1 | nq2 | nk2] -> collective
        pack = post.tile([128, 260], F32, tag="pack")
        nc.scalar.activation(out=pack[:, 0:128], in_=g_ps[:, 0, :],
                             func=AF.Copy)
        nc.scalar.activation(out=pack[:, 128:256], in_=g_ps[:, 1, :],
                             func=AF.Copy)
        nc.vector.tensor_scalar(pack[:, 256:258], nq2[:], 1.0, None, AL.mult)
        nc.vector.tensor_scalar(pack[:, 258:260], nk2[:], 1.0, None, AL.mult)

    cc_in = dram.tile([128, 260], F32)
    cc_out = dram.tile([128, 260], F32)
    nc.gpsimd.dma_start(out=cc_in[:, :], in_=pack[:])
    nc.gpsimd.collective_compute(
        "AllReduce", AL.add,
        replica_groups=[[0, 1], [2, 3], [4, 5], [6, 7]],
        ins=[cc_in.opt()], outs=[cc_out.opt()])

    red = post.tile([128, 260], F32, tag="red")
    nc.gpsimd.dma_start(out=red[:], in_=cc_out[:, :])

    # ================= phase C: softmax + Pp/f assembly ================
    with tc.tile_pool(name="pc", bufs=1, space="PSUM") as pc2:
        # inv norms
        invq = small.tile([128, 2], F32, tag="invq")
        invk = small.tile([128, 2], F32, tag="invk")
        for dst, src_off, mul_temp in ((invq, 256, True), (invk, 258, False)):
            sq_ = small.tile([128, 2], F32, tag="invn_sq")
            nc.scalar.activation(out=sq_[:], in_=red[:, src_off:src_off + 2],
                                 func=AF.Sqrt, bias=zb[:, :], scale=1.0)
            nc.vector.tensor_scalar_max(sq_[:], sq_[:], EPS_NORM)
            nc.vector.reciprocal(out=dst[:], in_=sq_[:])
            if mul_temp:
                nc.vector.tensor_tensor(out=dst[:], in0=dst[:],
                                        in1=temp_b[:, :], op=AL.mult)
        # invk col -> row broadcast via DRAM bounce
        ik_d = dram.tile([1, 256], F32)
        ikap = ik_d[:, :]
        nc.gpsimd.dma_start(out=bass.AP(tensor=ikap.tensor, offset=ikap.offset,
                                        ap=[[1, 128], [128, 2]]), in_=invk[:])
        ikb = post.tile([128, 256], F32, tag="ikb")
        nc.sync.dma_start(out=ikb[:], in_=_bcast(ik_d[0:1, :]))

        # softmax per head -> A_sb bf16
        a_sb = post.tile([128, 2, 128], BF, tag="a_sb")
        esc = post.tile([128, 2, 128], F32, tag="esc")
        for h in range(2):
            lh_t = post.tile([128, 128], F32, tag="lh_t")
            nc.vector.tensor_scalar(lh_t[:], red[:, h * 128:(h + 1) * 128],
                                    invq[:, h:h + 1], None, AL.mult)
            nc.vector.tensor_tensor(out=lh_t[:], in0=lh_t[:],
                                    in1=ikb[:, h * 128:(h + 1) * 128],
                                    op=AL.mult)
            rmax = small.tile([128, 1], F32, tag="rmax")
            nc.vector.tensor_reduce(out=rmax[:], in_=lh_t[:], op=AL.max,
                                    axis=AX.X)
            nc.vector.tensor_scalar(rmax[:], rmax[:], -1.0, None, AL.mult)
            rsum = small.tile([128, 1], F32, tag="rsum")
            nc.scalar.activation(out=esc[:, h, :], in_=lh_t[:], func=AF.Exp,
                                 bias=rmax[:, :], scale=1.0,
                                 accum_out=rsum[:])
            nc.vector.reciprocal(out=rsum[:], in_=rsum[:])
            nc.vector.tensor_scalar(a_sb[:, h, :], esc[:, h, :],
                                    rsum[:, :], None, AL.mult)

        # attn^T per head (for f-thin matmuls)
        tr2_ps = pc2.tile([128, 2, 128], BF, tag="tr2")
        attnT = post.tile([128, 2, 128], BF, tag="attnT")
        for h in range(2):
            nc.tensor.transpose(tr2_ps[:, h, :], a_sb[:, h, :], eye_sb[:])
        for h in range(2):
            nc.scalar.activation(out=attnT[:, h, :], in_=tr2_ps[:, h, :],
                                 func=AF.Copy)

        # E_h = Wo_h A_h  [p, o] ; then ET, PpT
        e_ps = pc2.tile([128, 2, 2, 128], F32, tag="e_ps")
        for h in range(2):
            for ph in range(2):
                nc.tensor.matmul(out=e_ps[:, ph, h, :],
                                 lhsT=woT[:, h, ph * 128:(ph + 1) * 128],
                                 rhs=a_sb[:, h, :], start=True, stop=True)
        e_sb = post.tile([128, 2, 2, 128], BF, tag="e_sb")
        for ph in range(2):
            nc.vector.tensor_scalar(e_sb[:, ph, 0, :], e_ps[:, ph, 0, :],
                                    1.0, None, AL.mult)
            nc.vector.tensor_scalar(e_sb[:, ph, 1, :], e_ps[:, ph, 1, :],
                                    1.0, None, AL.mult)
        et_ps = pc2.tile([128, 2, 2, 128], BF, tag="et_ps")
        eT_sb = post.tile([128, 2, 256], BF, tag="eT")
        for h in range(2):
            for ph in range(2):
                nc.tensor.transpose(et_ps[:, h, ph, :], e_sb[:, ph, h, :],
                                    eye_sb[:])
        for h in range(2):
            for ph in range(2):
                nc.scalar.activation(out=eT_sb[:, h, ph * 128:(ph + 1) * 128],
                                     in_=et_ps[:, h, ph, :], func=AF.Copy)
        ppt_ps = pc2.tile([128, 2, 256], F32, tag="ppt")
        for mh in range(2):
            for h in range(2):
                nc.tensor.matmul(out=ppt_ps[:, mh, :],
                                 lhsT=wvr[:, h, mh * 128:(mh + 1) * 128],
                                 rhs=eT_sb[:, h, :],
                                 start=(h == 0), stop=(h == 1))
        pptT = post.tile([128, 2, 256], BF, tag="pptT")
        nc.vector.tensor_scalar(pptT[:, 0, :], ppt_ps[:, 0, :], 1.0, None,
                                AL.mult)
        nc.vector.tensor_scalar(pptT[:, 1, :], ppt_ps[:, 1, :], 1.0, None,
                                AL.mult)

        # f1 = WoA sv, f2 = WoA bv2 + bo
        t_ps = pc2.tile([128, 2, 2], F32, tag="t_ps")
        for h in range(2):
            nc.tensor.matmul(out=t_ps[:, h, :], lhsT=attnT[:, h, :],
                             rhs=svbv[:, h, :], start=True, stop=True)
        t_sb = post.tile([128, 2, 2], BF, tag="t_sb")
        nc.vector.tensor_scalar(t_sb[:], t_ps[:], 1.0, None, AL.mult)
        f12_ps = pc2.tile([128, 2, 2], F32, tag="f12")
        for ph in range(2):
            for h in range(2):
                nc.tensor.matmul(out=f12_ps[:, ph, :],
                                 lhsT=woT[:, h, ph * 128:(ph + 1) * 128],
                                 rhs=t_sb[:, h, :],
                                 start=(h == 0), stop=(h == 1))
        f12_sb = post.tile([128, 2, 2], F32, tag="f12sb")
        nc.vector.tensor_scalar(f12_sb[:, :, 0], f12_ps[:, :, 0], 1.0, None,
                                AL.mult)
        nc.vector.tensor_tensor(out=f12_sb[:, :, 1], in0=f12_ps[:, :, 1],
                                in1=bo_col[:, :], op=AL.add)
        f1_d = dram.tile([1, 256], F32)
        f2_d = dram.tile([1, 256], F32)
        for k2, dd in ((0, f1_d), (1, f2_d)):
            ap = dd[:, :]
            nc.gpsimd.dma_start(out=bass.AP(tensor=ap.tensor, offset=ap.offset,
                                            ap=[[1, 128], [128, 2]]),
                                in_=f12_sb[:, :, k2])
        f1b = post.tile([128, 256], F32, tag="f1b")
        nc.sync.dma_start(out=f1b[:], in_=_bcast(f1_d[0:1, :]))
        f2row = post.tile([1, 256], BF, tag="f2row")
        nc.gpsimd.dma_start(out=f2row[:, :], in_=f2_d[0:1, :])

    # ================= phase D: output pass ================
    with tc.tile_pool(name="mm2", bufs=4, space="PSUM") as mm2:
        for g in range(NCH):
            j0 = g * CHT
            out_sb = outp.tile([128, CHT, 256], BF, tag="out_sb")
            for jj in range(CHT):
                j = j0 + jj
                opsum = mm2.tile([128, 256], F32, tag="opsum")
                nc.tensor.matmul(out=opsum[:], lhsT=rT_all[:, j, 0, :],
                                 rhs=pptT[:, 0, :], start=True, stop=False)
                nc.tensor.matmul(out=opsum[:], lhsT=rT_all[:, j, 1, :],
                                 rhs=pptT[:, 1, :], start=False, stop=False)
                nc.tensor.matmul(out=opsum[:], lhsT=ones_row[0:1, :],
                                 rhs=f2row[0:1, :], start=False, stop=True)
                nc.vector.scalar_tensor_tensor(
                    out=out_sb[:, jj, :], in0=f1b[:],
                    scalar=c_col[:, j:j + 1], op0=AL.mult, op1=AL.add,
                    in1=opsum[:])
            nc.sync.dma_start(out=outv[:, j0:j0 + CHT, :], in_=out_sb[:])


# ======================= host side =======================

def _prep_shared(inputs):
    f32 = np.float32
    Wq = np.asarray(inputs["Wq"], f32)
    bq = np.asarray(inputs["bq"], f32)
    Wkv = np.asarray(inputs["Wkv"], f32)
    bkv = np.asarray(inputs["bkv"], f32)
    Wo = np.asarray(inputs["Wo"], f32)
    bo = np.asarray(inputs["bo"], f32)
    lnS_w = np.asarray(inputs["lnS_w"], f32)
    lnS_b = np.asarray(inputs["lnS_b"], f32)
    lnR_w = np.asarray(inputs["lnR_w"], f32)
    lnR_b = np.asarray(inputs["lnR_b"], f32)
    temp = np.asarray(inputs["temperature"], f32).reshape(H)

    Wk, Wv = Wkv[:D], Wkv[D:]
    WqS = Wq * lnS_w[None, :]
    WkR = Wk * lnR_w[None, :]
    WvR = Wv * lnR_w[None, :]
    sq = WqS.sum(1)
    sk = WkR.sum(1)
    sv = WvR.sum(1)
    bq2 = Wq @ lnS_b + bq
    bk2 = Wk @ lnR_b + bkv[:D]
    bv2 = Wv @ lnR_b + bkv[D:]

    def halved(M):  # [256, 256] -> [128, 2, 256] with rows split in halves
        return np.ascontiguousarray(
            M.reshape(2, 128, M.shape[1]).transpose(1, 0, 2)).astype(BF16)

    def colh(v):
        return np.ascontiguousarray(v.reshape(2, 128).T, f32)

    svbv = np.stack([sv, bv2], 1)  # [256, 2]
    return {
        "wqsT": halved(np.ascontiguousarray(WqS.T)),
        "wkrT": halved(np.ascontiguousarray(WkR.T)),
        "wqs": halved(WqS),
        "wkr": halved(WkR),
        "wvr": halved(WvR),
        "woT": halved(np.ascontiguousarray(Wo.T)),
        "svbv": halved(svbv),
        "sq_col": colh(sq),
        "bq2_col": colh(bq2),
        "sk_col": colh(sk),
        "bk2_col": colh(bk2),
        "bo_col": colh(bo),
        "sk_row": sk.reshape(1, D).astype(f32),
        "bk2_row": bk2.reshape(1, D).astype(f32),
        "eye": np.eye(128).astype(BF16),
        "temp": temp.reshape(1, H).astype(f32),
    }


def _get_nc():
    if "nc" not in _nc_cache:
        _nc_cache["nc"] = _build_nc()
    return _nc_cache["nc"]


def run(inputs, trace=False):
    nc = _get_nc()
    shared = _prep_shared(inputs)
    iR = np.asarray(inputs["input_R"], np.float32)
    iS = np.asarray(inputs["input_S"], np.float32)
    in_maps = []
    for ci in range(N_CORES):
        b, half = ci // 2, ci % 2
        m = dict(shared)
        m["x_r"] = np.ascontiguousarray(iR[b, half * T:(half + 1) * T])
        m["x_s"] = np.ascontiguousarray(iS[b, half * T:(half + 1) * T])
        in_maps.append(m)
    res = run_bass_kernel_spmd(nc, in_maps, list(range(N_CORES)), trace=trace)
    out = np.zeros((B, N, D), np.float32)
    for ci in range(N_CORES):
        b, half = ci // 2, ci % 2
        out[b, half * T:(half + 1) * T] = np.asarray(
            res.results[ci]["out"]).astype(np.float32)
    return out, res


def kernel(**inputs):
    out, _ = run(inputs, trace=False)
    return out


# revision 12
# speedup vs baseline: 1.3307x; 1.2968x over previous
"""Channel-attention (XCA-style) Trainium2 kernel, 8-way SPMD — v3.

Shapes (hardcoded): B=4, N=16384, D=256, H=2 heads, c=128.
Sharding: core ci -> batch b=ci//2, token half ci%2 (T=8192 tokens/core).

C-matrix factorization: accumulate token-contraction Grams of the scaled
raw inputs (C_rs, C_ss, C_rr + moment vectors against [1, a_t, c_t]),
then assemble attention logits G and the q/k L2 norms as small [256x256]
weight products, pair-AllReduce only [128,260] f32, and collapse the
whole v/attn@v/Wo path into one matrix Pp applied per token from the
d-major transpose of the scaled kv input.

v3: chunk-level DMA transposes, per-chunk stats tiles + deep buffering
for pipelining, batched DRAM bounces, PE warmup chain across the
collective gap.
"""
import sys, types

sys.path.insert(0, "/opt/trn_rl_repo")

try:
    import antenv
    if "antenv.axon_hooks" not in sys.modules:
        _hooks = types.ModuleType("antenv.axon_hooks")
        _hooks._hook = None
        _hooks.set_axon_ntff_profile_hook = lambda h: setattr(_hooks, "_hook", h)
        _hooks.get_axon_ntff_profile_hook = lambda: _hooks._hook
        sys.modules["antenv.axon_hooks"] = _hooks
        antenv.axon_hooks = _hooks
        from trn_agent_boot.trn_boot import _ntff_profile_via_ctypes
        _hooks.set_axon_ntff_profile_hook(
            _ntff_profile_via_ctypes("/opt/axon/libaxon_pjrt.so"))
except Exception:
    pass

import numpy as np
import ml_dtypes

import concourse.bass as bass
import concourse.bacc as bacc
import concourse.mybir as mybir
import concourse.tile as tile
from concourse.bass_utils import run_bass_kernel_spmd

BF16 = ml_dtypes.bfloat16
F32 = mybir.dt.float32
BF = mybir.dt.bfloat16
AL = mybir.AluOpType
AF = mybir.ActivationFunctionType
AX = mybir.AxisListType

B, N, D, H = 4, 16384, 256, 2
T = N // 2                  # tokens per core
NT = 64                     # token tiles per core (inner j), token = p*64 + j
CHT = 8                     # tiles per chunk
NCH = NT // CHT             # 8 chunks
EPS_LN = 1e-5
EPS_NORM = 1e-12
N_CORES = 8
TCORE = float(T)

# stile column layout (bf16): [pad 0:13 | wcol 13:16 | s' 16:272]
WC0 = 13
SP0 = 16
SW = 272

_nc_cache = {}


def _bcast(ap, rows=128):
    return bass.AP(tensor=ap.tensor, offset=ap.offset,
                   ap=[[0, rows]] + [list(x) for x in ap.ap[1:]])


def _build_nc():
    nc = bacc.Bacc("TRN2", target_bir_lowering=False, debug=False,
                   num_devices=N_CORES)

    def ein(name, shape, dt=F32):
        return nc.dram_tensor(name, list(shape), dt, kind="ExternalInput")

    d_s = ein("x_s", [T, D])            # q source shard (input_S)
    d_r = ein("x_r", [T, D])            # kv source shard (input_R)
    d_wqsT = ein("wqsT", [128, 2, D], BF)
    d_wkrT = ein("wkrT", [128, 2, D], BF)
    d_wqs = ein("wqs", [128, 2, D], BF)
    d_wkr = ein("wkr", [128, 2, D], BF)
    d_wvr = ein("wvr", [128, 2, D], BF)
    d_woT = ein("woT", [128, 2, D], BF)
    d_svbv = ein("svbv", [128, 2, 2], BF)
    d_sqk4 = ein("sqk4", [128, 4])      # [sq_h0 sq_h1 sk_h0 sk_h1]
    d_bqk4 = ein("bqk4", [128, 4])      # [bq2 | bk2]
    d_boc = ein("bo_col", [128, 2])
    d_skr = ein("sk_row", [1, D])
    d_bk2r = ein("bk2_row", [1, D])
    d_eye = ein("eye", [128, 128], BF)
    d_temp = ein("temp", [1, 2])
    d_out = nc.dram_tensor("out", [T, D], BF, kind="ExternalOutput")

    svw = d_s.rearrange("(p j) d -> p j d", p=128)
    rvw = d_r.rearrange("(p j) d -> p j d", p=128)
    outv = d_out.rearrange("(p j) d -> p j d", p=128)

    with tile.TileContext(nc) as tc:
        import contextlib
        with contextlib.ExitStack() as ctx:
            _body(ctx, tc, nc, svw, rvw, outv, d_wqsT, d_wkrT, d_wqs, d_wkr,
                  d_wvr, d_woT, d_svbv, d_sqk4, d_bqk4, d_boc, d_skr, d_bk2r,
                  d_eye, d_temp)
    nc.finalize()
    return nc


def _body(ctx, tc, nc, svw, rvw, outv, d_wqsT, d_wkrT, d_wqs, d_wkr, d_wvr,
          d_woT, d_svbv, d_sqk4, d_bqk4, d_boc, d_skr, d_bk2r, d_eye, d_temp):
    E = ctx.enter_context
    consts = E(tc.tile_pool(name="consts", bufs=1))
    stats = E(tc.tile_pool(name="stats", bufs=1))
    ldp = E(tc.tile_pool(name="ldp", bufs=3))
    cbp = E(tc.tile_pool(name="cbp", bufs=3))
    small = E(tc.tile_pool(name="small", bufs=4))
    pers = E(tc.tile_pool(name="pers", bufs=1))
    post = E(tc.tile_pool(name="post", bufs=1))
    outp = E(tc.tile_pool(name="outp", bufs=2))
    dram = E(tc.tile_pool(name="dram", bufs=1, space="DRAM"))

    # ---------------- constants ----------------
    wqsT = consts.tile([128, 2, D], BF, tag="wqsT")
    wkrT = consts.tile([128, 2, D], BF, tag="wkrT")
    wqs = consts.tile([128, 2, D], BF, tag="wqs")
    wkr = consts.tile([128, 2, D], BF, tag="wkr")
    wvr = consts.tile([128, 2, D], BF, tag="wvr")
    woT = consts.tile([128, 2, D], BF, tag="woT")
    for dst, src in ((wqsT, d_wqsT), (wkrT, d_wkrT), (wqs, d_wqs),
                     (wkr, d_wkr), (wvr, d_wvr), (woT, d_woT)):
        nc.sync.dma_start(out=dst[:], in_=src[:, :, :])
    svbv = consts.tile([128, 2, 2], BF, tag="svbv")
    nc.sync.dma_start(out=svbv[:], in_=d_svbv[:, :, :])
    sqk4 = consts.tile([128, 4], F32, tag="sqk4")
    bqk4 = consts.tile([128, 4], F32, tag="bqk4")
    bo_col = consts.tile([128, 2], F32, tag="boc")
    for dst, src in ((sqk4, d_sqk4), (bqk4, d_bqk4), (bo_col, d_boc)):
        nc.sync.dma_start(out=dst[:], in_=src[:, :])
    skrow_b = consts.tile([128, D], F32, tag="skrb")
    bk2row_b = consts.tile([128, D], F32, tag="bk2rb")
    nc.sync.dma_start(out=skrow_b[:], in_=_bcast(d_skr[:, :]))
    nc.sync.dma_start(out=bk2row_b[:], in_=_bcast(d_bk2r[:, :]))
    eye_sb = consts.tile([128, 128], BF, tag="eye")
    nc.sync.dma_start(out=eye_sb[:], in_=d_eye[:, :])
    temp_b = consts.tile([128, 2], F32, tag="tempb")
    nc.sync.dma_start(out=temp_b[:], in_=_bcast(d_temp[:, :]))
    ones_row = consts.tile([1, 128], BF, tag="ones")
    nc.vector.memset(ones_row[:], 1.0)
    epsln = consts.tile([128, 1], F32, tag="epsln")
    nc.vector.memset(epsln[:], EPS_LN)
    zb = consts.tile([128, 1], F32, tag="zb")
    nc.vector.memset(zb[:], 0.0)

    c_col = stats.tile([128, NT], F32, tag="c_col")
    sqscr = stats.tile([128, 256], BF, tag="sqscr")   # ACT square scratch
    sqscr2 = stats.tile([128, 256], BF, tag="sqscr2")  # DVE square scratch
    rT_all = pers.tile([128, NT, 2, 128], BF, tag="rT")

    # ================= phase A: stream chunks =================
    with tc.tile_pool(name="accA", bufs=1, space="PSUM") as accA:
        b_rs0 = accA.tile([128, 259], F32, tag="b_rs0")
        b_rs1 = accA.tile([128, 259], F32, tag="b_rs1")
        b_ss0 = accA.tile([128, 259], F32, tag="b_ss0")
        b_ss1 = accA.tile([128, 259], F32, tag="b_ss1")
        b_rr0 = accA.tile([128, 256], F32, tag="b_rr0")
        b_rr1 = accA.tile([128, 256], F32, tag="b_rr1")
        b_wg = accA.tile([128, 3], F32, tag="b_wg")

        for ch in range(NCH):
            j0 = ch * CHT
            s_raw = ldp.tile([128, CHT, D], BF, tag="s_raw")
            r_raw = ldp.tile([128, CHT, D], BF, tag="r_raw")
            nc.gpsimd.dma_start(out=s_raw[:], in_=svw[:, j0:j0 + CHT, :])
            nc.gpsimd.dma_start(out=r_raw[:], in_=rvw[:, j0:j0 + CHT, :])
            stile = cbp.tile([128, CHT, SW], BF, tag="stile")
            rtile = cbp.tile([128, CHT * 256], BF, tag="rtile")

            # --- per-token LN stats ---
            sums_s = small.tile([128, CHT], F32, tag="sums_s")
            sums_r = small.tile([128, CHT], F32, tag="sums_r")
            ssq_s = small.tile([128, CHT], F32, tag="ssq_s")
            ssq_r = small.tile([128, CHT], F32, tag="ssq_r")
            nc.vector.tensor_reduce(out=sums_s[:], in_=s_raw[:], axis=AX.X,
                                    op=AL.add)
            nc.vector.tensor_reduce(out=sums_r[:], in_=r_raw[:], axis=AX.X,
                                    op=AL.add)
            for jj in range(CHT):
                nc.scalar.activation(out=sqscr[:], in_=s_raw[:, jj, :],
                                     func=AF.Square,
                                     accum_out=ssq_s[:, jj:jj + 1])
                if jj % 2 == 0:
                    nc.scalar.activation(out=sqscr[:], in_=r_raw[:, jj, :],
                                         func=AF.Square,
                                         accum_out=ssq_r[:, jj:jj + 1])
                else:
                    nc.vector.scalar_tensor_tensor(
                        out=sqscr2[:], in0=r_raw[:, jj, :], scalar=0.0,
                        op0=AL.bypass, op1=AL.mult, in1=r_raw[:, jj, :],
                        accum_out=ssq_r[:, jj:jj + 1])

            for inp in range(2):
                sums = sums_s if inp == 0 else sums_r
                ssq = ssq_s if inp == 0 else ssq_r
                mu = small.tile([128, CHT], F32, tag="mu")
                nc.vector.tensor_scalar(mu[:], sums[:], 1.0 / D, None, AL.mult)
                musq = small.tile([128, CHT], F32, tag="musq")
                nc.scalar.activation(out=musq[:], in_=mu[:], func=AF.Square)
                var = small.tile([128, CHT], F32, tag="var")
                nc.vector.scalar_tensor_tensor(
                    out=var[:], in0=ssq[:], scalar=1.0 / D,
                    op0=AL.mult, op1=AL.subtract, in1=musq[:])
                sig = small.tile([128, CHT], F32, tag="sig")
                nc.scalar.activation(out=sig[:], in_=var[:], func=AF.Sqrt,
                                     bias=epsln[:, :], scale=1.0)
                invs = small.tile([128, CHT], F32, tag="invs")
                nc.vector.reciprocal(out=invs[:], in_=sig[:])
                if inp == 0:
                    nc.vector.scalar_tensor_tensor(
                        out=stile[:, :, WC0 + 1], in0=mu[:], scalar=-1.0,
                        op0=AL.mult, op1=AL.mult, in1=invs[:])
                    for jj in range(CHT):
                        nc.vector.tensor_scalar(
                            stile[:, jj, SP0:SP0 + 256], s_raw[:, jj, :],
                            invs[:, jj:jj + 1], None, AL.mult)
                else:
                    nc.vector.scalar_tensor_tensor(
                        out=c_col[:, j0:j0 + CHT], in0=mu[:], scalar=-1.0,
                        op0=AL.mult, op1=AL.mult, in1=invs[:])
                    nc.scalar.activation(out=stile[:, :, WC0 + 2],
                                         in_=c_col[:, j0:j0 + CHT],
                                         func=AF.Copy)
                    for jj in range(CHT):
                        nc.scalar.activation(
                            out=rtile[:, jj * 256:(jj + 1) * 256],
                            in_=r_raw[:, jj, :], func=AF.Copy, bias=0.0,
                            scale=invs[:, jj:jj + 1])
            nc.gpsimd.memset(stile[:, :, WC0], 1.0)

            # --- accumulation matmuls (7/tile) + one chunk transpose ---
            for jj in range(CHT):
                j = j0 + jj
                st = (j == 0)
                sp = (j == NT - 1)
                rhs_ws = stile[:, jj, WC0:SP0 + 256]     # [wcol | s'] 259
                rhs_r = rtile[:, jj * 256:(jj + 1) * 256]
                for h in range(2):
                    nc.tensor.matmul(
                        out=(b_rs0 if h == 0 else b_rs1)[:],
                        lhsT=rtile[:, jj * 256 + h * 128:jj * 256 + (h + 1) * 128],
                        rhs=rhs_ws, start=st, stop=sp)
                for h in range(2):
                    nc.tensor.matmul(
                        out=(b_ss0 if h == 0 else b_ss1)[:],
                        lhsT=stile[:, jj, SP0 + h * 128:SP0 + (h + 1) * 128],
                        rhs=rhs_ws, start=st, stop=sp)
                for h in range(2):
                    nc.tensor.matmul(
                        out=(b_rr0 if h == 0 else b_rr1)[:],
                        lhsT=rtile[:, jj * 256 + h * 128:jj * 256 + (h + 1) * 128],
                        rhs=rhs_r, start=st, stop=sp)
                nc.tensor.matmul(out=b_wg[0:3, :],
                                 lhsT=stile[:, jj, WC0:WC0 + 3],
                                 rhs=stile[:, jj, WC0:WC0 + 3],
                                 start=st, stop=sp)
            nc.sync.dma_start_transpose(rT_all[:, j0:j0 + CHT, :, :],
                                        rtile[:])

        # ---- evac C matrices (bf16) + S-gram ----
        crs_sb = post.tile([128, 2, 259], BF, tag="crs")
        css_sb = post.tile([128, 2, 259], BF, tag="css")
        crr_sb = post.tile([128, 2, 256], BF, tag="crr")
        sg_sb = post.tile([3, 3], F32, tag="sg")
        nc.vector.tensor_scalar(crs_sb[:, 0, :], b_rs0[:], 1.0, None, AL.mult)
        nc.vector.tensor_scalar(crs_sb[:, 1, :], b_rs1[:], 1.0, None, AL.mult)
        nc.scalar.activation(out=css_sb[:, 0, :], in_=b_ss0[:], func=AF.Copy)
        nc.scalar.activation(out=css_sb[:, 1, :], in_=b_ss1[:], func=AF.Copy)
        nc.vector.tensor_scalar(crr_sb[:, 0, :], b_rr0[:], 1.0, None, AL.mult)
        nc.scalar.activation(out=crr_sb[:, 1, :], in_=b_rr1[:], func=AF.Copy)
        nc.vector.tensor_scalar(sg_sb[:], b_wg[0:3, :], 1.0, None, AL.mult)

    # S-gram row broadcast via one DRAM bounce: [3,3] -> [1,9] -> [128,9]
    sg_d = dram.tile([1, 9], F32)
    sgap = sg_d[:, :]
    nc.gpsimd.dma_start(out=bass.AP(tensor=sgap.tensor, offset=sgap.offset,
                                    ap=[[3, 3], [1, 3]]), in_=sg_sb[:])
    sgb = post.tile([128, 9], F32, tag="sgb")
    nc.sync.dma_start(out=sgb[:], in_=_bcast(sg_d[0:1, :]))
    # col indices in sgb: Sa=1, Sc=2, Saa=4, Sac=5, Scc=8

    with tc.tile_pool(name="pb", bufs=1, space="PSUM") as pb:
        th4 = pb.tile([128, 4, 3], F32, tag="th4")  # q:[beta|eps|alpha] k:[delta|gam|zeta]
        xh_ps = pb.tile([128, 2, 256], F32, tag="xh")
        g_ps = pb.tile([128, 2, 128], F32, tag="g")
        z_ps = pb.tile([128, 2, 256], F32, tag="z")
        tr_ps = pb.tile([128, 2, 128], BF, tag="tr")

        for ih in range(2):
            for lh in range(2):
                nc.tensor.matmul(out=th4[:, ih, :],
                                 lhsT=wqsT[:, lh, ih * 128:(ih + 1) * 128],
                                 rhs=css_sb[:, lh, 0:3],
                                 start=(lh == 0), stop=(lh == 1))
                nc.tensor.matmul(out=th4[:, 2 + ih, :],
                                 lhsT=wkrT[:, lh, ih * 128:(ih + 1) * 128],
                                 rhs=crs_sb[:, lh, 0:3],
                                 start=(lh == 0), stop=(lh == 1))
        # Xk_h = Wk_h C_rs   [o in h, j_s(256)]
        for h in range(2):
            for lh in range(2):
                nc.tensor.matmul(out=xh_ps[:, h, :],
                                 lhsT=wkrT[:, lh, h * 128:(h + 1) * 128],
                                 rhs=crs_sb[:, lh, 3:259],
                                 start=(lh == 0), stop=(lh == 1))
        x_sb = post.tile([128, 2, 256], BF, tag="x_sb")
        nc.vector.tensor_scalar(x_sb[:, 0, :], xh_ps[:, 0, :], 1.0, None,
                                AL.mult)
        nc.scalar.activation(out=x_sb[:, 1, :], in_=xh_ps[:, 1, :],
                             func=AF.Copy)
        xT_sb = post.tile([128, 2, 2, 128], BF, tag="xT")
        for h in range(2):
            for jh in range(2):
                nc.tensor.transpose(tr_ps[:, jh, :],
                                    x_sb[:, h, jh * 128:(jh + 1) * 128],
                                    eye_sb[:])
            nc.vector.tensor_scalar(xT_sb[:, h, 0, :], tr_ps[:, 0, :], 1.0,
                                    None, AL.mult)
            nc.scalar.activation(out=xT_sb[:, h, 1, :], in_=tr_ps[:, 1, :],
                                 func=AF.Copy)
        # G_h[i,o] = sum_js WqS[i,js] XkT[js,o]
        for h in range(2):
            for jh in range(2):
                nc.tensor.matmul(out=g_ps[:, h, :],
                                 lhsT=wqsT[:, jh, h * 128:(h + 1) * 128],
                                 rhs=xT_sb[:, h, jh, :],
                                 start=(jh == 0), stop=(jh == 1))
        # d4 = [dq | dk] diag terms
        d4 = small.tile([128, 4], F32, tag="d4")
        dscr = post.tile([128, 256], F32, tag="dscr")
        for a in range(2):
            for lh in range(2):
                nc.tensor.matmul(out=z_ps[:, a, :],
                                 lhsT=wqsT[:, lh, a * 128:(a + 1) * 128],
                                 rhs=css_sb[:, lh, 3:259],
                                 start=(lh == 0), stop=(lh == 1))
        for a in range(2):
            nc.vector.scalar_tensor_tensor(
                out=dscr[:], in0=z_ps[:, a, :], scalar=0.0, op0=AL.bypass,
                op1=AL.mult, in1=wqs[:, a, :], accum_out=d4[:, a:a + 1])
        for a in range(2):
            for lh in range(2):
                nc.tensor.matmul(out=z_ps[:, a, :],
                                 lhsT=wkrT[:, lh, a * 128:(a + 1) * 128],
                                 rhs=crr_sb[:, lh, :],
                                 start=(lh == 0), stop=(lh == 1))
        for a in range(2):
            nc.vector.scalar_tensor_tensor(
                out=dscr[:], in0=z_ps[:, a, :], scalar=0.0, op0=AL.bypass,
                op1=AL.mult, in1=wkr[:, a, :], accum_out=d4[:, 2 + a:3 + a])

        # norms (q and k combined on [128,4]):
        # n = d + 2*th[...,1]*u + 2*th[...,0]*g + sXX*u^2 + 2*sX*u*g + T*g^2
        nqk2 = small.tile([128, 4], F32, tag="nqk2")
        t1 = small.tile([128, 4], F32, tag="t1")
        t2 = small.tile([128, 4], F32, tag="t2")
        sXX4 = small.tile([128, 4], F32, tag="sXX4")
        sX4 = small.tile([128, 4], F32, tag="sX4")
        for cdst, csrc in ((sXX4[:, 0:2], 4), (sXX4[:, 2:4], 8),
                           (sX4[:, 0:2], 1), (sX4[:, 2:4], 2)):
            nc.vector.tensor_scalar(cdst, _bcfree(sgb, csrc, 2), 1.0, None,
                                    AL.mult)
        nc.vector.tensor_tensor(out=t1[:, 0:2], in0=th4[:, 0:2, 1],
                                in1=sqk4[:, 0:2], op=AL.mult)
        nc.vector.tensor_tensor(out=t1[:, 2:4], in0=th4[:, 2:4, 2],
                                in1=sqk4[:, 2:4], op=AL.mult)
        nc.vector.scalar_tensor_tensor(out=nqk2[:], in0=t1[:], scalar=2.0,
                                       op0=AL.mult, op1=AL.add, in1=d4[:])
        nc.vector.tensor_tensor(out=t1[:], in0=th4[:, :, 0], in1=bqk4[:],
                                op=AL.mult)
        nc.vector.scalar_tensor_tensor(out=nqk2[:], in0=t1[:], scalar=2.0,
                                       op0=AL.mult, op1=AL.add, in1=nqk2[:])
        nc.vector.tensor_tensor(out=t1[:], in0=sqk4[:], in1=sqk4[:],
                                op=AL.mult)
        nc.vector.tensor_tensor(out=t2[:], in0=t1[:], in1=sXX4[:], op=AL.mult)
        nc.vector.tensor_tensor(out=nqk2[:], in0=nqk2[:], in1=t2[:], op=AL.add)
        nc.vector.tensor_tensor(out=t1[:], in0=sqk4[:], in1=bqk4[:],
                                op=AL.mult)
        nc.vector.tensor_tensor(out=t2[:], in0=t1[:], in1=sX4[:], op=AL.mult)
        nc.vector.scalar_tensor_tensor(out=nqk2[:], in0=t2[:], scalar=2.0,
                                       op0=AL.mult, op1=AL.add, in1=nqk2[:])
        nc.vector.tensor_tensor(out=t1[:], in0=bqk4[:], in1=bqk4[:],
                                op=AL.mult)
        nc.vector.scalar_tensor_tensor(out=nqk2[:], in0=t1[:], scalar=TCORE,
                                       op0=AL.mult, op1=AL.add, in1=nqk2[:])

        # G rank-1 rows (k-side combos) -> one DRAM bounce for both rows
        r12c = small.tile([128, 2, 2], F32, tag="r12c")  # [m(row1/2), h]
        nc.vector.scalar_tensor_tensor(out=r12c[:, 0, :], in0=sqk4[:, 2:4],
                                       scalar=sgb[:, 5:6], op0=AL.mult,
                                       op1=AL.add, in1=th4[:, 2:4, 1])
        nc.vector.scalar_tensor_tensor(out=r12c[:, 0, :], in0=bqk4[:, 2:4],
                                       scalar=sgb[:, 1:2], op0=AL.mult,
                                       op1=AL.add, in1=r12c[:, 0, :])
        nc.vector.scalar_tensor_tensor(out=r12c[:, 1, :], in0=sqk4[:, 2:4],
                                       scalar=sgb[:, 2:3], op0=AL.mult,
                                       op1=AL.add, in1=th4[:, 2:4, 0])
        nc.vector.scalar_tensor_tensor(out=r12c[:, 1, :], in0=bqk4[:, 2:4],
                                       scalar=TCORE, op0=AL.mult,
                                       op1=AL.add, in1=r12c[:, 1, :])
        r12_d = dram.tile([2, 256], F32)
        rdap = r12_d[:, :]
        nc.gpsimd.dma_start(
            out=bass.AP(tensor=rdap.tensor, offset=rdap.offset,
                        ap=[[1, 128], [256, 2], [128, 2]]),
            in_=r12c[:])
        r12_b = post.tile([128, 2, 256], F32, tag="r12b")
        nc.sync.dma_start(out=r12_b[:], in_=bass.AP(
            tensor=rdap.tensor, offset=rdap.offset,
            ap=[[0, 128], [256, 2], [1, 256]]))
        for h in range(2):
            gh = g_ps[:, h, :]
            nc.vector.scalar_tensor_tensor(
                out=gh, in0=skrow_b[:, h * 128:(h + 1) * 128],
                scalar=th4[:, h, 2:3], op0=AL.mult, op1=AL.add, in1=gh)
            nc.vector.scalar_tensor_tensor(
                out=gh, in0=bk2row_b[:, h * 128:(h + 1) * 128],
                scalar=th4[:, h, 0:1], op0=AL.mult, op1=AL.add, in1=gh)
            nc.vector.scalar_tensor_tensor(
                out=gh, in0=r12_b[:, 0, h * 128:(h + 1) * 128],
                scalar=sqk4[:, h:h + 1], op0=AL.mult, op1=AL.add, in1=gh)
            nc.vector.scalar_tensor_tensor(
                out=gh, in0=r12_b[:, 1, h * 128:(h + 1) * 128],
                scalar=bqk4[:, h:h + 1], op0=AL.mult, op1=AL.add, in1=gh)

        # pack [G0 | G1 | nq2 | nk2]
        pack = post.tile([128, 260], F32, tag="pack")
        nc.scalar.activation(out=pack[:, 0:128], in_=g_ps[:, 0, :],
                             func=AF.Copy)
        nc.scalar.activation(out=pack[:, 128:256], in_=g_ps[:, 1, :],
                             func=AF.Copy)
        nc.vector.tensor_scalar(pack[:, 256:260], nqk2[:], 1.0, None, AL.mult)

    cc_in = dram.tile([128, 260], F32)
    cc_out = dram.tile([128, 260], F32)
    nc.gpsimd.dma_start(out=cc_in[:, :], in_=pack[:])
    nc.gpsimd.collective_compute(
        "AllReduce", AL.add,
        replica_groups=[[0, 1], [2, 3], [4, 5], [6, 7]],
        ins=[cc_in.opt()], outs=[cc_out.opt()])

    red = post.tile([128, 260], F32, tag="red")
    nc.gpsimd.dma_start(out=red[:], in_=cc_out[:, :])

    # ================= phase C: softmax + Pp/f assembly ================
    with tc.tile_pool(name="pc", bufs=1, space="PSUM") as pc2:
        # --- PE warmup chain to keep HAM hot across the collective gap ---
        wu_ps = pc2.tile([128, 128], F32, tag="wu_ps")
        wu_sb = post.tile([128, 128], BF, tag="wu_sb")
        nc.vector.tensor_scalar(wu_sb[:], eye_sb[:], 1.0, None, AL.mult)
        for k in range(10):
            nc.tensor.matmul(out=wu_ps[:], lhsT=wu_sb[:], rhs=eye_sb[:],
                             start=True, stop=True)
            nc.vector.tensor_scalar(wu_sb[:], wu_ps[:], 1.0, None, AL.mult)

        invq = small.tile([128, 2], F32, tag="invq")
        invk = small.tile([128, 2], F32, tag="invk")
        for dst, src_off, mul_temp in ((invq, 256, True), (invk, 258, False)):
            sq_ = small.tile([128, 2], F32, tag="invn_sq")
            nc.scalar.activation(out=sq_[:], in_=red[:, src_off:src_off + 2],
                                 func=AF.Sqrt, bias=zb[:, :], scale=1.0)
            nc.vector.tensor_scalar_max(sq_[:], sq_[:], EPS_NORM)
            nc.vector.reciprocal(out=dst[:], in_=sq_[:])
            if mul_temp:
                nc.vector.tensor_tensor(out=dst[:], in0=dst[:],
                                        in1=temp_b[:, :], op=AL.mult)
        ik_d = dram.tile([1, 256], F32)
        ikap = ik_d[:, :]
        nc.gpsimd.dma_start(out=bass.AP(tensor=ikap.tensor, offset=ikap.offset,
                                        ap=[[1, 128], [128, 2]]), in_=invk[:])
        ikb = post.tile([128, 256], F32, tag="ikb")
        nc.sync.dma_start(out=ikb[:], in_=_bcast(ik_d[0:1, :]))

        a_sb = post.tile([128, 2, 128], BF, tag="a_sb")
        esc = post.tile([128, 2, 128], F32, tag="esc")
        for h in range(2):
            lh_t = post.tile([128, 128], F32, tag="lh_t")
            nc.vector.tensor_scalar(lh_t[:], red[:, h * 128:(h + 1) * 128],
                                    invq[:, h:h + 1], None, AL.mult)
            nc.vector.tensor_tensor(out=lh_t[:], in0=lh_t[:],
                                    in1=ikb[:, h * 128:(h + 1) * 128],
                                    op=AL.mult)
            rmax = small.tile([128, 1], F32, tag="rmax")
            nc.vector.tensor_reduce(out=rmax[:], in_=lh_t[:], op=AL.max,
                                    axis=AX.X)
            nc.vector.tensor_scalar(rmax[:], rmax[:], -1.0, None, AL.mult)
            rsum = small.tile([128, 1], F32, tag="rsum")
            nc.scalar.activation(out=esc[:, h, :], in_=lh_t[:], func=AF.Exp,
                                 bias=rmax[:, :], scale=1.0,
                                 accum_out=rsum[:])
            nc.vector.reciprocal(out=rsum[:], in_=rsum[:])
            nc.vector.tensor_scalar(a_sb[:, h, :], esc[:, h, :],
                                    rsum[:, :], None, AL.mult)

        tr2_ps = pc2.tile([128, 2, 128], BF, tag="tr2")
        attnT = post.tile([128, 2, 128], BF, tag="attnT")
        for h in range(2):
            nc.tensor.transpose(tr2_ps[:, h, :], a_sb[:, h, :], eye_sb[:])
        for h in range(2):
            nc.scalar.activation(out=attnT[:, h, :], in_=tr2_ps[:, h, :],
                                 func=AF.Copy)

        e_ps = pc2.tile([128, 2, 2, 128], F32, tag="e_ps")
        for h in range(2):
            for ph in range(2):
                nc.tensor.matmul(out=e_ps[:, ph, h, :],
                                 lhsT=woT[:, h, ph * 128:(ph + 1) * 128],
                                 rhs=a_sb[:, h, :], start=True, stop=True)
        e_sb = post.tile([128, 2, 2, 128], BF, tag="e_sb")
        for ph in range(2):
            nc.vector.tensor_scalar(e_sb[:, ph, 0, :], e_ps[:, ph, 0, :],
                                    1.0, None, AL.mult)
            nc.scalar.activation(out=e_sb[:, ph, 1, :], in_=e_ps[:, ph, 1, :],
                                 func=AF.Copy)
        et_ps = pc2.tile([128, 2, 2, 128], BF, tag="et_ps")
        eT_sb = post.tile([128, 2, 256], BF, tag="eT")
        for h in range(2):
            for ph in range(2):
                nc.tensor.transpose(et_ps[:, h, ph, :], e_sb[:, ph, h, :],
                                    eye_sb[:])
        for h in range(2):
            nc.vector.tensor_scalar(eT_sb[:, h, 0:128], et_ps[:, h, 0, :],
                                    1.0, None, AL.mult)
            nc.scalar.activation(out=eT_sb[:, h, 128:256],
                                 in_=et_ps[:, h, 1, :], func=AF.Copy)
        ppt_ps = pc2.tile([128, 2, 256], F32, tag="ppt")
        for mh in range(2):
            for h in range(2):
                nc.tensor.matmul(out=ppt_ps[:, mh, :],
                                 lhsT=wvr[:, h, mh * 128:(mh + 1) * 128],
                                 rhs=eT_sb[:, h, :],
                                 start=(h == 0), stop=(h == 1))
        pptT = post.tile([128, 2, 256], BF, tag="pptT")
        nc.vector.tensor_scalar(pptT[:, 0, :], ppt_ps[:, 0, :], 1.0, None,
                                AL.mult)
        nc.scalar.activation(out=pptT[:, 1, :], in_=ppt_ps[:, 1, :],
                             func=AF.Copy)

        t_ps = pc2.tile([128, 2, 2], F32, tag="t_ps")
        for h in range(2):
            nc.tensor.matmul(out=t_ps[:, h, :], lhsT=attnT[:, h, :],
                             rhs=svbv[:, h, :], start=True, stop=True)
        t_sb = post.tile([128, 2, 2], BF, tag="t_sb")
        nc.vector.tensor_scalar(t_sb[:], t_ps[:], 1.0, None, AL.mult)
        f12_ps = pc2.tile([128, 2, 2], F32, tag="f12")
        for ph in range(2):
            for h in range(2):
                nc.tensor.matmul(out=f12_ps[:, ph, :],
                                 lhsT=woT[:, h, ph * 128:(ph + 1) * 128],
                                 rhs=t_sb[:, h, :],
                                 start=(h == 0), stop=(h == 1))
        f12_sb = post.tile([128, 2, 2], F32, tag="f12sb")
        nc.vector.tensor_scalar(f12_sb[:, :, 0], f12_ps[:, :, 0], 1.0, None,
                                AL.mult)
        nc.vector.tensor_tensor(out=f12_sb[:, :, 1], in0=f12_ps[:, :, 1],
                                in1=bo_col[:, :], op=AL.add)
        f_d = dram.tile([2, 256], F32)
        fdap = f_d[:, :]
        for m in range(2):
            nc.gpsimd.dma_start(
                out=bass.AP(tensor=fdap.tensor, offset=fdap.offset + m * 256,
                            ap=[[1, 128], [128, 2]]),
                in_=f12_sb[:, :, m])
        f1b = post.tile([128, 256], F32, tag="f1b")
        nc.sync.dma_start(out=f1b[:], in_=_bcast(f_d[0:1, :]))
        f2row = post.tile([1, 256], BF, tag="f2row")
        nc.gpsimd.dma_start(out=f2row[:, :], in_=f_d[1:2, :])

    # ================= phase D: output pass ================
    with tc.tile_pool(name="mm2", bufs=4, space="PSUM") as mm2:
        for g in range(NCH):
            j0 = g * CHT
            out_sb = outp.tile([128, CHT, 256], BF, tag="out_sb")
            for jj in range(CHT):
                j = j0 + jj
                opsum = mm2.tile([128, 256], F32, tag="opsum")
                nc.tensor.matmul(out=opsum[:], lhsT=rT_all[:, j, 0, :],
                                 rhs=pptT[:, 0, :], start=True, stop=False)
                nc.tensor.matmul(out=opsum[:], lhsT=rT_all[:, j, 1, :],
                                 rhs=pptT[:, 1, :], start=False, stop=False)
                nc.tensor.matmul(out=opsum[:], lhsT=ones_row[0:1, :],
                                 rhs=f2row[0:1, :], start=False, stop=True)
                nc.vector.scalar_tensor_tensor(
                    out=out_sb[:, jj, :], in0=f1b[:],
                    scalar=c_col[:, j:j + 1], op0=AL.mult, op1=AL.add,
                    in1=opsum[:])
            nc.sync.dma_start(out=outv[:, j0:j0 + CHT, :], in_=out_sb[:])


def _bcfree(tile_, col, n):
    """AP reading tile_[:, col] broadcast n times along free (0-stride)."""
    ap = tile_[:, col:col + 1]
    return bass.AP(tensor=ap.tensor, offset=ap.offset,
                   ap=[list(ap.ap[0])] + [[0, n]])


# ======================= host side =======================

def _prep_shared(inputs):
    f32 = np.float32
    Wq = np.asarray(inputs["Wq"], f32)
    bq = np.asarray(inputs["bq"], f32)
    Wkv = np.asarray(inputs["Wkv"], f32)
    bkv = np.asarray(inputs["bkv"], f32)
    Wo = np.asarray(inputs["Wo"], f32)
    bo = np.asarray(inputs["bo"], f32)
    lnS_w = np.asarray(inputs["lnS_w"], f32)
    lnS_b = np.asarray(inputs["lnS_b"], f32)
    lnR_w = np.asarray(inputs["lnR_w"], f32)
    lnR_b = np.asarray(inputs["lnR_b"], f32)
    temp = np.asarray(inputs["temperature"], f32).reshape(H)

    Wk, Wv = Wkv[:D], Wkv[D:]
    WqS = Wq * lnS_w[None, :]
    WkR = Wk * lnR_w[None, :]
    WvR = Wv * lnR_w[None, :]
    sq = WqS.sum(1)
    sk = WkR.sum(1)
    sv = WvR.sum(1)
    bq2 = Wq @ lnS_b + bq
    bk2 = Wk @ lnR_b + bkv[:D]
    bv2 = Wv @ lnR_b + bkv[D:]

    def halved(M):  # [256, X] -> [128, 2, X] rows split into halves
        return np.ascontiguousarray(
            M.reshape(2, 128, M.shape[1]).transpose(1, 0, 2)).astype(BF16)

    def colh(v):
        return np.ascontiguousarray(v.reshape(2, 128).T, f32)

    svbv = np.stack([sv, bv2], 1)  # [256, 2]
    sqk4 = np.concatenate([colh(sq), colh(sk)], 1)
    bqk4 = np.concatenate([colh(bq2), colh(bk2)], 1)
    return {
        "wqsT": halved(np.ascontiguousarray(WqS.T)),
        "wkrT": halved(np.ascontiguousarray(WkR.T)),
        "wqs": halved(WqS),
        "wkr": halved(WkR),
        "wvr": halved(WvR),
        "woT": halved(np.ascontiguousarray(Wo.T)),
        "svbv": halved(svbv),
        "sqk4": np.ascontiguousarray(sqk4),
        "bqk4": np.ascontiguousarray(bqk4),
        "bo_col": colh(bo),
        "sk_row": sk.reshape(1, D).astype(f32),
        "bk2_row": bk2.reshape(1, D).astype(f32),
        "eye": np.eye(128).astype(BF16),
        "temp": temp.reshape(1, H).astype(f32),
    }


def _get_nc():
    if "nc" not in _nc_cache:
        _nc_cache["nc"] = _build_nc()
    return _nc_cache["nc"]


def run(inputs, trace=False):
    nc = _get_nc()
    shared = _prep_shared(inputs)
    iR = np.asarray(inputs["input_R"], np.float32)
    iS = np.asarray(inputs["input_S"], np.float32)
    in_maps = []
    for ci in range(N_CORES):
        b, half = ci // 2, ci % 2
        m = dict(shared)
        m["x_r"] = np.ascontiguousarray(iR[b, half * T:(half + 1) * T])
        m["x_s"] = np.ascontiguousarray(iS[b, half * T:(half + 1) * T])
        in_maps.append(m)
    res = run_bass_kernel_spmd(nc, in_maps, list(range(N_CORES)), trace=trace)
    out = np.zeros((B, N, D), np.float32)
    for ci in range(N_CORES):
        b, half = ci // 2, ci % 2
        out[b, half * T:(half + 1) * T] = np.asarray(
            res.results[ci]["out"]).astype(np.float32)
    return out, res


def kernel(**inputs):
    out, _ = run(inputs, trace=False)
    return out
